# revision 5
# baseline (speedup 1.0000x reference)
"""Multi-head self-attention (B=4, S=2048, D=1024, H=16) on 8 TRN2 NeuronCores.

Sharding: core i = (batch b = i//2, head-group g = i%2). Each core computes,
for its batch and its 8 heads: QKV projection, attention, and a partial
output projection over its 512 attention features. Host sums the two
partials per batch (Megatron-style tensor parallel over heads x data
parallel over batch).

V2 restructure vs the first working version:
  - Single pass over x: V, K, Q projections all computed per s-chunk from
    one set of x tiles (x was previously DMA'd twice and the Q/K weight
    DMAs stalled the tensor engine ~30us mid-kernel waiting for SBUF).
  - All projection weights prefetched into SBUF up front, in parallel
    with the first x chunk.
  - Q and K are stored in bf16 (halves their SBUF footprint so the
    single-pass layout fits; scores matmul in bf16 costs the same PE
    cycles as fp32r but lets one matmul stream 1024 moving columns).
  - Scores for a head-pair run as ONE matmul over the fused [qa|qb]
    [128, 1024] bf16 moving operand (zero-padded per head half as
    before), halving score instruction count.
  - fp8 was evaluated and rejected: attention output is a near-complete
    cancellation sum, so per-element fp8 quantization error (~4%)
    survives as ~3-5e-2 relative output error (gate is 2e-2). bf16 q/k
    measures 1.7e-3 end to end.

Per-core dataflow (transposed orientation so the softmax denominator
comes out of the PE array for free):
  V[t,e]   = x^T-stationary matmuls over Wv^T + ones column per head
  K^T[f,s] = Wk-stationary matmuls over x^T (bf16, full tensor resident)
  Q^T[f,s] = Wq-stationary, stored zero-padded per head half in qt2
             (head-A rows live / B rows zero | A rows zero / B live)
  S^T[t,s] = K^T-tile-stationary matmul against qt2 (both heads, one
             1024-col moving pass)
  P^T      = exp(S^T / 8) (ScalarE, PSUM->SBUF, no max-sub: fp32 exp of
             scores ~N(0,16) pre-scale is safe)
  O^T_aug  = V_aug-stationary matmuls over P^T (M=65); row 64 = denom
  On = O^T * recip(denom); out^T = Wout^T-stationary over On.
"""
import os
import sys
import types

import numpy as np

# ---------------------------------------------------------------------------
# environment bootstrap (self-contained: no problem-dir imports)
# ---------------------------------------------------------------------------


def _install_ntff_hook():
    """run_bass_kernel_spmd(trace=True) under axon needs antenv.axon_hooks,
    which the agent image's antenv stub lacks. Recreate it."""
    if "antenv.axon_hooks" in sys.modules:
        return
    try:
        import antenv
        from trn_agent_boot.trn_boot import _ntff_profile_via_ctypes
    except Exception:
        return
    so_path = "/opt/axon/libaxon_pjrt.so"
    if not os.path.exists(so_path):
        return
    mod = types.ModuleType("antenv.axon_hooks")
    _hook = [_ntff_profile_via_ctypes(so_path)]
    mod.get_axon_ntff_profile_hook = lambda: _hook[0]

    def _set(h):
        _hook[0] = h

    mod.set_axon_ntff_profile_hook = _set
    sys.modules["antenv.axon_hooks"] = mod
    antenv.axon_hooks = mod


_install_ntff_hook()

import concourse.bacc as bacc
import concourse.tile as tile
from concourse import mybir
from concourse.bass_utils import run_bass_kernel_spmd
from contextlib import ExitStack

# ---------------------------------------------------------------------------
# problem constants (hardcoded per contract)
# ---------------------------------------------------------------------------
B, S, D = 4, 2048, 1024
H, HD = 16, 64
HPG = 8            # heads per core (group)
E = HPG * HD       # 512 attention features per core
P = 128
SC = 512           # s-chunk
NS = S // SC       # 4 s-chunks
NT = S // P        # 16 t-chunks
ND = D // P        # 8 d-chunks
NF = E // P        # 4 f-chunks per Q (or K) = head-pairs
HD1 = HD + 1       # V_aug columns per head (V + ones)
SCALE = 1.0 / np.sqrt(np.float32(HD))

F32 = mybir.dt.float32
F32R = mybir.dt.float32r
BF16 = mybir.dt.bfloat16
EXP = mybir.ActivationFunctionType.Exp

FUSED_SCORES = False  # 1024-col moving fails walrus s3d3_mm_num_elements

_NC_CACHE = {}


def _build_nc():
    nc = bacc.Bacc("TRN2", target_bir_lowering=False)

    xT = nc.dram_tensor("xT", [D, S], F32R, kind="ExternalInput")
    wqT = nc.dram_tensor("wqT", [D, E], F32R, kind="ExternalInput")
    wkT = nc.dram_tensor("wkT", [D, E], F32R, kind="ExternalInput")
    wvT = nc.dram_tensor("wvT", [D, E], F32R, kind="ExternalInput")
    woT = nc.dram_tensor("woT", [E, D], F32R, kind="ExternalInput")
    bq = nc.dram_tensor("bq", [E, 1], F32, kind="ExternalInput")
    bk = nc.dram_tensor("bk", [E, 1], F32, kind="ExternalInput")
    bv = nc.dram_tensor("bv", [1, E], F32, kind="ExternalInput")
    bo = nc.dram_tensor("bo", [D, 1], F32, kind="ExternalInput")
    outT = nc.dram_tensor("outT", [D, S], F32, kind="ExternalOutput")

    with tile.TileContext(nc) as tc, ExitStack() as glob:
        const = glob.enter_context(tc.tile_pool(name="const", bufs=1))
        bv_bc = const.tile([P, E], F32, name="bv_bc")
        resid = glob.enter_context(tc.tile_pool(name="resid", bufs=1))
        # qt2[f]: [128, NS*2*SC] bf16, layout (s, half, col): half 0 holds
        # head-A rows 0-63 (rows 64-127 zero), half 1 the opposite.
        qt2 = [resid.tile([P, NS * 2 * SC], BF16, name=f"qt2_{f}") for f in range(NF)]
        kt = [resid.tile([P, S], BF16, name=f"kt{f}") for f in range(NF)]
        vt = [resid.tile([P, HPG * HD1], F32R, name=f"vt{t}") for t in range(NT)]
        for f in range(NF):
            qv = qt2[f][:].rearrange("p (s h c) -> p s h c", h=2, c=SC)
            nc.vector.memset(qv[HD:P, :, 0, :], 0.0)
            nc.vector.memset(qv[0:HD, :, 1, :], 0.0)

        # ---------------- phase 1: single-pass QKV projection -----------
        with ExitStack() as c1:
            wpool = c1.enter_context(tc.tile_pool(name="w", bufs=1))
            wv = [wpool.tile([P, E], F32R, name=f"wv{d}") for d in range(ND)]
            wk = [wpool.tile([P, E], F32R, name=f"wk{d}") for d in range(ND)]
            wq = [wpool.tile([P, E], F32R, name=f"wq{d}") for d in range(ND)]
            xpool = c1.enter_context(tc.tile_pool(name="x", bufs=2))
            # prefetch order: wv + x(s0) first (first matmuls), then wk/wq
            for d in range(ND):
                nc.sync.dma_start(wv[d][:], wvT[d * P:(d + 1) * P, :])
            xts0 = [xpool.tile([P, SC], F32R, name="xts", tag=f"x{d}")
                    for d in range(ND)]
            for d in range(ND):
                nc.sync.dma_start(xts0[d][:], xT[d * P:(d + 1) * P, 0:SC])
            for d in range(ND):
                nc.sync.dma_start(wk[d][:], wkT[d * P:(d + 1) * P, :])
            for d in range(ND):
                nc.sync.dma_start(wq[d][:], wqT[d * P:(d + 1) * P, :])
            nc.sync.dma_start(bv_bc[:], bv[0:1, :].to_broadcast((P, E)))
            bqt = [wpool.tile([P, 1], F32, name=f"bqt{f}") for f in range(NF)]
            bkt = [wpool.tile([P, 1], F32, name=f"bkt{f}") for f in range(NF)]
            for f in range(NF):
                nc.sync.dma_start(bqt[f][:], bq[f * P:(f + 1) * P, :])
                nc.sync.dma_start(bkt[f][:], bk[f * P:(f + 1) * P, :])

            psv = c1.enter_context(tc.tile_pool(name="psv", bufs=2, space="PSUM"))
            psq = c1.enter_context(tc.tile_pool(name="psq", bufs=4, space="PSUM"))

            for s in range(NS):
                sl = slice(s * SC, (s + 1) * SC)
                if s == 0:
                    xts = xts0
                else:
                    xts = [xpool.tile([P, SC], F32R, name="xts", tag=f"x{d}")
                           for d in range(ND)]
                    for d in range(ND):
                        nc.sync.dma_start(xts[d][:], xT[d * P:(d + 1) * P, sl])
                # V: x-stationary, stream Wv (out [t, 512 feats])
                for i in range(NS):
                    t = s * NS + i
                    ps = psv.tile([P, E], F32, name="psvt", tag="psv")
                    for d in range(ND):
                        nc.tensor.matmul(
                            ps[:], xts[d][:, i * P:(i + 1) * P], wv[d][:],
                            start=(d == 0), stop=(d == ND - 1))
                    vdst = vt[t][:].rearrange("p (h c) -> p h c", c=HD1)
                    nc.vector.tensor_add(
                        vdst[:, :, 0:HD],
                        ps[:].rearrange("p (h c) -> p h c", c=HD),
                        bv_bc[:].rearrange("p (h c) -> p h c", c=HD))
                    nc.vector.memset(vdst[:, :, HD:HD1].bitcast(F32), 1.0)
                # K: Wk-stationary, stream x (out [kfeat, 512 s-cols])
                for f in range(NF):
                    ps = psq.tile([P, SC], F32, name="pskt", tag="psq")
                    for d in range(ND):
                        nc.tensor.matmul(
                            ps[:], wk[d][:, f * P:(f + 1) * P], xts[d][:],
                            start=(d == 0), stop=(d == ND - 1))
                    nc.vector.tensor_scalar_add(kt[f][:, sl], ps[:], bkt[f][:])
                # Q: same, evicted zero-padded per half into qt2
                for f in range(NF):
                    ps = psq.tile([P, SC], F32, name="psqt", tag="psq")
                    for d in range(ND):
                        nc.tensor.matmul(
                            ps[:], wq[d][:, f * P:(f + 1) * P], xts[d][:],
                            start=(d == 0), stop=(d == ND - 1))
                    nc.vector.tensor_scalar_add(
                        qt2[f][0:HD, (2 * s) * SC:(2 * s + 1) * SC],
                        ps[0:HD, :], bqt[f][0:HD, :])
                    nc.vector.tensor_scalar_add(
                        qt2[f][HD:P, (2 * s + 1) * SC:(2 * s + 2) * SC],
                        ps[HD:P, :], bqt[f][HD:P, :])

        # ---------------- phase 2: attention + out-proj -----------------
        with ExitStack() as c2:
            wo_pool = c2.enter_context(tc.tile_pool(name="wo", bufs=1))
            wo = [wo_pool.tile([P, D], F32R, name=f"wo{e}") for e in range(NF)]
            for e in range(NF):
                nc.sync.dma_start(wo[e][:], woT[e * P:(e + 1) * P, :])
            bot = [wo_pool.tile([P, 1], F32, name=f"bot{i}") for i in range(ND)]
            for i in range(ND):
                nc.sync.dma_start(bot[i][:], bo[i * P:(i + 1) * P, :])

            dram_pool = c2.enter_context(tc.tile_pool(name="dramrs", bufs=2, space="DRAM"))
            pt_pool = c2.enter_context(tc.tile_pool(name="pt", bufs=4))
            on_pool = c2.enter_context(tc.tile_pool(name="on", bufs=2))
            rs_pool = c2.enter_context(tc.tile_pool(name="rs", bufs=2))
            rb_pool = c2.enter_context(tc.tile_pool(name="rb", bufs=2))
            ot_pool = c2.enter_context(tc.tile_pool(name="ot", bufs=3))
            ps_sc = c2.enter_context(tc.tile_pool(name="ps_sc", bufs=2, space="PSUM"))
            ps_o = c2.enter_context(tc.tile_pool(name="ps_o", bufs=1, space="PSUM"))
            ps_op = c2.enter_context(tc.tile_pool(name="ps_op", bufs=2, space="PSUM"))

            for s in range(NS):
                sl = slice(s * SC, (s + 1) * SC)
                on_tiles = [on_pool.tile([P, SC], F32R, name="on", tag=f"on{hp}")
                            for hp in range(NF)]
                for hp in range(NF):
                    hA, hB = 2 * hp, 2 * hp + 1
                    o_psA = ps_o.tile([P, SC], F32, name="opsA", tag="oA")
                    o_psB = ps_o.tile([P, SC], F32, name="opsB", tag="oB")
                    for t in range(NT):
                        tsl = slice(t * P, (t + 1) * P)
                        sc_ps = ps_sc.tile([P, 2 * SC], F32, name="scps", tag="sc")
                        if FUSED_SCORES:
                            nc.tensor.matmul(
                                sc_ps[:],
                                kt[hp][:, tsl],
                                qt2[hp][:, (2 * s) * SC:(2 * s + 2) * SC],
                                start=True, stop=True)
                        else:
                            nc.tensor.matmul(
                                sc_ps[:, 0:SC], kt[hp][:, tsl],
                                qt2[hp][:, (2 * s) * SC:(2 * s + 1) * SC],
                                start=True, stop=True)
                            nc.tensor.matmul(
                                sc_ps[:, SC:2 * SC], kt[hp][:, tsl],
                                qt2[hp][:, (2 * s + 1) * SC:(2 * s + 2) * SC],
                                start=True, stop=True)
                        pt = pt_pool.tile([P, 2 * SC], F32R, name="ptile", tag="pt")
                        nc.scalar.activation(pt[:], sc_ps[:], EXP, scale=float(SCALE))
                        # PV with ones column: out rows 0-63 = O^T, row 64 = sums
                        nc.tensor.matmul(
                            o_psA[0:HD1, :],
                            vt[t][:, hA * HD1:(hA + 1) * HD1],
                            pt[:, 0:SC],
                            start=(t == 0), stop=(t == NT - 1))
                        nc.tensor.matmul(
                            o_psB[0:HD1, :],
                            vt[t][:, hB * HD1:(hB + 1) * HD1],
                            pt[:, SC:2 * SC],
                            start=(t == 0), stop=(t == NT - 1))
                    # evict O_aug to SBUF quickly (frees PSUM for next head),
                    # then normalize off the critical path: broadcast the raw
                    # sums row via a DRAM bounce and divide on DVE.
                    ocA = rs_pool.tile([P, SC], F32, name="ocA", tag="ocA")
                    ocB = rs_pool.tile([P, SC], F32, name="ocB", tag="ocB")
                    nc.vector.tensor_copy(ocA[0:HD1, :], o_psA[0:HD1, :])
                    nc.vector.tensor_copy(ocB[0:HD1, :], o_psB[0:HD1, :])
                    # reciprocal of the two sums rows on all 128 DVE lanes:
                    # bounce each [1,512] row through DRAM, reload as [64,8]
                    # partition-spread, one reciprocal, bounce back.
                    rd = dram_pool.tile([2, SC], F32, name="rdtile", tag="rd")
                    nc.sync.dma_start(rd[0:1, :], ocA[HD:HD1, :])
                    nc.sync.dma_start(rd[1:2, :], ocB[HD:HD1, :])
                    rsp = rs_pool.tile([P, SC // HD], F32, name="rsp", tag="rsp")
                    nc.sync.dma_start(
                        rsp[0:HD, :],
                        rd[0:1, :].rearrange("a (p c) -> (a p) c", c=SC // HD))
                    nc.sync.dma_start(
                        rsp[HD:P, :],
                        rd[1:2, :].rearrange("a (p c) -> (a p) c", c=SC // HD))
                    nc.vector.reciprocal(rsp[:], rsp[:])
                    nc.sync.dma_start(
                        rd[0:1, :].rearrange("a (p c) -> (a p) c", c=SC // HD),
                        rsp[0:HD, :])
                    nc.sync.dma_start(
                        rd[1:2, :].rearrange("a (p c) -> (a p) c", c=SC // HD),
                        rsp[HD:P, :])
                    rb = rb_pool.tile([HD, SC], F32, name="rbtile", tag="rb")
                    rb2 = rb_pool.tile([HD, SC], F32, name="rb2tile", tag="rb2")
                    nc.sync.dma_start(rb[0:HD, :], rd[0:1, :].to_broadcast((HD, SC)))
                    nc.sync.dma_start(rb2[0:HD, :], rd[1:2, :].to_broadcast((HD, SC)))
                    # head A -> partitions 0-63 directly; head B -> via SBUF
                    # tmp then a DMA partition-shift to 64-127
                    nc.vector.tensor_mul(
                        on_tiles[hp][0:HD, :], ocA[0:HD, :], rb[0:HD, :])
                    tmpB = rb_pool.tile([HD, SC], F32R, name="tmpB", tag="tmpB")
                    nc.vector.tensor_mul(
                        tmpB[0:HD, :], ocB[0:HD, :], rb2[0:HD, :])
                    nc.sync.dma_start(on_tiles[hp][HD:P, :], tmpB[0:HD, :])
                # output projection for this s-chunk
                for dc in range(ND):
                    op_ps = ps_op.tile([P, SC], F32, name="opps", tag="op")
                    for e in range(NF):
                        nc.tensor.matmul(
                            op_ps[:], wo[e][:, dc * P:(dc + 1) * P], on_tiles[e][:],
                            start=(e == 0), stop=(e == NF - 1))
                    ot = ot_pool.tile([P, SC], F32, name="ottile", tag="ot")
                    nc.vector.tensor_scalar_add(ot[:], op_ps[:], bot[dc][:])
                    nc.sync.dma_start(outT[dc * P:(dc + 1) * P, sl], ot[:])

    nc.finalize()
    return nc


def _get_nc():
    if "nc" not in _NC_CACHE:
        _NC_CACHE["nc"] = _build_nc()
    return _NC_CACHE["nc"]


def _shard_inputs(x, w_qkv, b_qkv, w_out, b_out):
    """Build the 8 per-core input maps. Core i = (b = i//2, g = i%2)."""
    x = np.asarray(x, np.float32)
    w_qkv = np.asarray(w_qkv, np.float32)
    b_qkv = np.asarray(b_qkv, np.float32)
    w_out = np.asarray(w_out, np.float32)
    b_out = np.asarray(b_out, np.float32)

    in_maps = []
    for b in range(B):
        xT = np.ascontiguousarray(x[b].T)  # [D, S]
        for g in range(2):
            heads = range(g * HPG, (g + 1) * HPG)
            # w_qkv rows for head h: [192h, 192h+64) = Q, +64..128 = K, +128..192 = V
            q_rows = np.concatenate([np.arange(3 * HD * h, 3 * HD * h + HD) for h in heads])
            k_rows = q_rows + HD
            v_rows = q_rows + 2 * HD
            wqT = np.ascontiguousarray(w_qkv[q_rows].T)  # [D, E]
            wkT = np.ascontiguousarray(w_qkv[k_rows].T)
            wvT = np.ascontiguousarray(w_qkv[v_rows].T)
            ecols = np.arange(g * E, (g + 1) * E)
            woT = np.ascontiguousarray(w_out[:, ecols].T)  # [E, D]
            bo = b_out[:, None] if g == 0 else np.zeros((D, 1), np.float32)
            in_maps.append({
                "xT": xT,
                "wqT": wqT,
                "wkT": wkT,
                "wvT": wvT,
                "woT": woT,
                "bq": np.ascontiguousarray(b_qkv[q_rows][:, None]),
                "bk": np.ascontiguousarray(b_qkv[k_rows][:, None]),
                "bv": np.ascontiguousarray(b_qkv[v_rows][None, :]),
                "bo": np.ascontiguousarray(bo),
            })
    return in_maps


def run(inputs, trace=False):
    """Run the kernel; returns (full_output, exec_time_ns or None)."""
    nc = _get_nc()
    in_maps = _shard_inputs(**inputs)
    res = run_bass_kernel_spmd(nc, in_maps, core_ids=list(range(8)), trace=trace)
    out = np.empty((B, S, D), np.float32)
    for b in range(B):
        acc = res.results[2 * b]["outT"] + res.results[2 * b + 1]["outT"]
        out[b] = acc.T
    return out, res.exec_time_ns


def kernel(x, w_qkv, b_qkv, w_out, b_out):
    out, _ = run(dict(x=x, w_qkv=w_qkv, b_qkv=b_qkv, w_out=w_out, b_out=b_out))
    return out


# revision 16
# speedup vs baseline: 1.0214x; 1.0214x over previous
"""Multi-head self-attention (B=4, S=2048, D=1024, H=16) on 8 TRN2 NeuronCores.

Sharding: core i = (batch b = i//2, head-group g = i%2). Each core computes,
for its batch and its 8 heads: QKV projection, attention, and a partial
output projection over its 512 attention features. Host sums the two
partials per batch (Megatron-style tensor parallel over heads x data
parallel over batch).

V2 restructure vs the first working version:
  - Single pass over x: V, K, Q projections all computed per s-chunk from
    one set of x tiles (x was previously DMA'd twice and the Q/K weight
    DMAs stalled the tensor engine ~30us mid-kernel waiting for SBUF).
  - All projection weights prefetched into SBUF up front, in parallel
    with the first x chunk.
  - Q and K are stored in bf16 (halves their SBUF footprint so the
    single-pass layout fits; scores matmul in bf16 costs the same PE
    cycles as fp32r but lets one matmul stream 1024 moving columns).
  - Scores for a head-pair run as ONE matmul over the fused [qa|qb]
    [128, 1024] bf16 moving operand (zero-padded per head half as
    before), halving score instruction count.
  - fp8 was evaluated and rejected: attention output is a near-complete
    cancellation sum, so per-element fp8 quantization error (~4%)
    survives as ~3-5e-2 relative output error (gate is 2e-2). bf16 q/k
    measures 1.7e-3 end to end.

Per-core dataflow (transposed orientation so the softmax denominator
comes out of the PE array for free):
  V[t,e]   = x^T-stationary matmuls over Wv^T + ones column per head
  K^T[f,s] = Wk-stationary matmuls over x^T (bf16, full tensor resident)
  Q^T[f,s] = Wq-stationary, stored zero-padded per head half in qt2
             (head-A rows live / B rows zero | A rows zero / B live)
  S^T[t,s] = K^T-tile-stationary matmul against qt2 (both heads, one
             1024-col moving pass)
  P^T      = exp(S^T / 8) (ScalarE, PSUM->SBUF, no max-sub: fp32 exp of
             scores ~N(0,16) pre-scale is safe)
  O^T_aug  = V_aug-stationary matmuls over P^T (M=65); row 64 = denom
  On = O^T * recip(denom); out^T = Wout^T-stationary over On.
"""
import os
import sys
import types

import ml_dtypes
import numpy as np

# ---------------------------------------------------------------------------
# environment bootstrap (self-contained: no problem-dir imports)
# ---------------------------------------------------------------------------


def _install_ntff_hook():
    """run_bass_kernel_spmd(trace=True) under axon needs antenv.axon_hooks,
    which the agent image's antenv stub lacks. Recreate it."""
    if "antenv.axon_hooks" in sys.modules:
        return
    try:
        import antenv
        from trn_agent_boot.trn_boot import _ntff_profile_via_ctypes
    except Exception:
        return
    so_path = "/opt/axon/libaxon_pjrt.so"
    if not os.path.exists(so_path):
        return
    mod = types.ModuleType("antenv.axon_hooks")
    _hook = [_ntff_profile_via_ctypes(so_path)]
    mod.get_axon_ntff_profile_hook = lambda: _hook[0]

    def _set(h):
        _hook[0] = h

    mod.set_axon_ntff_profile_hook = _set
    sys.modules["antenv.axon_hooks"] = mod
    antenv.axon_hooks = mod


_install_ntff_hook()

import concourse.bacc as bacc
import concourse.tile as tile
from concourse import mybir
from concourse.bass_utils import run_bass_kernel_spmd
from contextlib import ExitStack

# ---------------------------------------------------------------------------
# problem constants (hardcoded per contract)
# ---------------------------------------------------------------------------
B, S, D = 4, 2048, 1024
H, HD = 16, 64
HPG = 8            # heads per core (group)
E = HPG * HD       # 512 attention features per core
P = 128
SC = 512           # s-chunk
NS = S // SC       # 4 s-chunks
NT = S // P        # 16 t-chunks
ND = D // P        # 8 d-chunks
NF = E // P        # 4 f-chunks per Q (or K) = head-pairs
HD1 = HD + 1       # V_aug columns per head (V + ones)
SCALE = 1.0 / np.sqrt(np.float32(HD))

F32 = mybir.dt.float32
F32R = mybir.dt.float32r
BF16 = mybir.dt.bfloat16
EXP = mybir.ActivationFunctionType.Exp

FUSED_SCORES = False  # 1024-col moving fails walrus s3d3_mm_num_elements

_NC_CACHE = {}


def _build_nc():
    nc = bacc.Bacc("TRN2", target_bir_lowering=False)

    xT = nc.dram_tensor("xT", [D, S], BF16, kind="ExternalInput")
    wqT = nc.dram_tensor("wqT", [D, E], BF16, kind="ExternalInput")
    wkT = nc.dram_tensor("wkT", [D, E], BF16, kind="ExternalInput")
    wvT = nc.dram_tensor("wvT", [D, E], BF16, kind="ExternalInput")
    woT = nc.dram_tensor("woT", [E, D], F32R, kind="ExternalInput")
    bq = nc.dram_tensor("bq", [E, 1], F32, kind="ExternalInput")
    bk = nc.dram_tensor("bk", [E, 1], F32, kind="ExternalInput")
    bv = nc.dram_tensor("bv", [1, E], F32, kind="ExternalInput")
    bo = nc.dram_tensor("bo", [D, 1], F32, kind="ExternalInput")
    outT = nc.dram_tensor("outT", [D, S], F32, kind="ExternalOutput")

    with tile.TileContext(nc) as tc, ExitStack() as glob:
        const = glob.enter_context(tc.tile_pool(name="const", bufs=1))
        bv_bc = const.tile([P, E], F32, name="bv_bc")
        resid = glob.enter_context(tc.tile_pool(name="resid", bufs=1))
        # qt2[f]: [128, NS*2*SC] bf16, layout (s, half, col): half 0 holds
        # head-A rows 0-63 (rows 64-127 zero), half 1 the opposite.
        qt2 = [resid.tile([P, NS * 2 * SC], BF16, name=f"qt2_{f}") for f in range(NF)]
        kt = [resid.tile([P, S], BF16, name=f"kt{f}") for f in range(NF)]
        vt = [resid.tile([P, HPG * HD1], F32R, name=f"vt{t}") for t in range(NT)]
        for f in range(NF):
            qv = qt2[f][:].rearrange("p (s h c) -> p s h c", h=2, c=SC)
            nc.vector.memset(qv[HD:P, :, 0, :], 0.0)
            nc.vector.memset(qv[0:HD, :, 1, :], 0.0)

        # ---------------- phase 1: single-pass QKV projection -----------
        with ExitStack() as c1:
            wpool = c1.enter_context(tc.tile_pool(name="w", bufs=1))
            wv = [wpool.tile([P, E], BF16, name=f"wv{d}") for d in range(ND)]
            wk = [wpool.tile([P, E], BF16, name=f"wk{d}") for d in range(ND)]
            wq = [wpool.tile([P, E], BF16, name=f"wq{d}") for d in range(ND)]
            xpool = c1.enter_context(tc.tile_pool(name="x", bufs=2))
            # prefetch order: wv + x(s0) first (first matmuls), then wk/wq
            for d in range(ND):
                nc.sync.dma_start(wv[d][:], wvT[d * P:(d + 1) * P, :])
            xts0 = [xpool.tile([P, SC], BF16, name="xts", tag=f"x{d}")
                    for d in range(ND)]
            for d in range(ND):
                nc.sync.dma_start(xts0[d][:], xT[d * P:(d + 1) * P, 0:SC])
            for d in range(ND):
                nc.sync.dma_start(wk[d][:], wkT[d * P:(d + 1) * P, :])
            for d in range(ND):
                nc.sync.dma_start(wq[d][:], wqT[d * P:(d + 1) * P, :])
            nc.sync.dma_start(bv_bc[:], bv[0:1, :].to_broadcast((P, E)))
            bqt = [wpool.tile([P, 1], F32, name=f"bqt{f}") for f in range(NF)]
            bkt = [wpool.tile([P, 1], F32, name=f"bkt{f}") for f in range(NF)]
            for f in range(NF):
                nc.sync.dma_start(bqt[f][:], bq[f * P:(f + 1) * P, :])
                nc.sync.dma_start(bkt[f][:], bk[f * P:(f + 1) * P, :])

            psv = c1.enter_context(tc.tile_pool(name="psv", bufs=2, space="PSUM"))
            psq = c1.enter_context(tc.tile_pool(name="psq", bufs=4, space="PSUM"))

            for s in range(NS):
                sl = slice(s * SC, (s + 1) * SC)
                if s == 0:
                    xts = xts0
                else:
                    xts = [xpool.tile([P, SC], BF16, name="xts", tag=f"x{d}")
                           for d in range(ND)]
                    for d in range(ND):
                        nc.sync.dma_start(xts[d][:], xT[d * P:(d + 1) * P, sl])
                # V: x-stationary, stream Wv (out [t, 512 feats])
                for i in range(NS):
                    t = s * NS + i
                    ps = psv.tile([P, E], F32, name="psvt", tag="psv")
                    for d in range(ND):
                        nc.tensor.matmul(
                            ps[:], xts[d][:, i * P:(i + 1) * P], wv[d][:],
                            start=(d == 0), stop=(d == ND - 1))
                    vdst = vt[t][:].rearrange("p (h c) -> p h c", c=HD1)
                    nc.vector.tensor_add(
                        vdst[:, :, 0:HD],
                        ps[:].rearrange("p (h c) -> p h c", c=HD),
                        bv_bc[:].rearrange("p (h c) -> p h c", c=HD))
                    nc.vector.memset(vdst[:, :, HD:HD1].bitcast(F32), 1.0)
                # K: Wk-stationary, stream x (out [kfeat, 512 s-cols])
                for f in range(NF):
                    ps = psq.tile([P, SC], F32, name="pskt", tag="psq")
                    for d in range(ND):
                        nc.tensor.matmul(
                            ps[:], wk[d][:, f * P:(f + 1) * P], xts[d][:],
                            start=(d == 0), stop=(d == ND - 1))
                    nc.vector.tensor_scalar_add(kt[f][:, sl], ps[:], bkt[f][:])
                # Q: same, evicted zero-padded per half into qt2
                for f in range(NF):
                    ps = psq.tile([P, SC], F32, name="psqt", tag="psq")
                    for d in range(ND):
                        nc.tensor.matmul(
                            ps[:], wq[d][:, f * P:(f + 1) * P], xts[d][:],
                            start=(d == 0), stop=(d == ND - 1))
                    nc.vector.tensor_scalar_add(
                        qt2[f][0:HD, (2 * s) * SC:(2 * s + 1) * SC],
                        ps[0:HD, :], bqt[f][0:HD, :])
                    nc.vector.tensor_scalar_add(
                        qt2[f][HD:P, (2 * s + 1) * SC:(2 * s + 2) * SC],
                        ps[HD:P, :], bqt[f][HD:P, :])

        # ---------------- phase 2: attention + out-proj -----------------
        with ExitStack() as c2:
            wo_pool = c2.enter_context(tc.tile_pool(name="wo", bufs=1))
            wo = [wo_pool.tile([P, D], F32R, name=f"wo{e}") for e in range(NF)]
            for e in range(NF):
                nc.sync.dma_start(wo[e][:], woT[e * P:(e + 1) * P, :])
            bot = [wo_pool.tile([P, 1], F32, name=f"bot{i}") for i in range(ND)]
            for i in range(ND):
                nc.sync.dma_start(bot[i][:], bo[i * P:(i + 1) * P, :])

            dram_pool = c2.enter_context(tc.tile_pool(name="dramrs", bufs=3, space="DRAM"))
            pt_pool = c2.enter_context(tc.tile_pool(name="pt", bufs=6))
            on_pool = c2.enter_context(tc.tile_pool(name="on", bufs=2))
            rs_pool = c2.enter_context(tc.tile_pool(name="rs", bufs=3))
            rb_pool = c2.enter_context(tc.tile_pool(name="rb", bufs=3))
            ot_pool = c2.enter_context(tc.tile_pool(name="ot", bufs=4))
            ps_sc = c2.enter_context(tc.tile_pool(name="ps_sc", bufs=2, space="PSUM"))
            ps_o = c2.enter_context(tc.tile_pool(name="ps_o", bufs=1, space="PSUM"))
            ps_op = c2.enter_context(tc.tile_pool(name="ps_op", bufs=2, space="PSUM"))

            for s in range(NS):
                sl = slice(s * SC, (s + 1) * SC)
                on_tiles = [on_pool.tile([P, SC], F32R, name="on", tag=f"on{hp}")
                            for hp in range(NF)]
                for hp in range(NF):
                    hA, hB = 2 * hp, 2 * hp + 1
                    o_psA = ps_o.tile([P, SC], F32, name="opsA", tag="oA")
                    o_psB = ps_o.tile([P, SC], F32, name="opsB", tag="oB")
                    for t in range(NT):
                        tsl = slice(t * P, (t + 1) * P)
                        sc_ps = ps_sc.tile([P, 2 * SC], F32, name="scps", tag="sc")
                        if FUSED_SCORES:
                            nc.tensor.matmul(
                                sc_ps[:],
                                kt[hp][:, tsl],
                                qt2[hp][:, (2 * s) * SC:(2 * s + 2) * SC],
                                start=True, stop=True)
                        else:
                            nc.tensor.matmul(
                                sc_ps[:, 0:SC], kt[hp][:, tsl],
                                qt2[hp][:, (2 * s) * SC:(2 * s + 1) * SC],
                                start=True, stop=True)
                            nc.tensor.matmul(
                                sc_ps[:, SC:2 * SC], kt[hp][:, tsl],
                                qt2[hp][:, (2 * s + 1) * SC:(2 * s + 2) * SC],
                                start=True, stop=True)
                        pt = pt_pool.tile([P, 2 * SC], F32R, name="ptile", tag="pt")
                        nc.scalar.activation(pt[:], sc_ps[:], EXP, scale=float(SCALE))
                        # PV with ones column: out rows 0-63 = O^T, row 64 = sums
                        nc.tensor.matmul(
                            o_psA[0:HD1, :],
                            vt[t][:, hA * HD1:(hA + 1) * HD1],
                            pt[:, 0:SC],
                            start=(t == 0), stop=(t == NT - 1))
                        nc.tensor.matmul(
                            o_psB[0:HD1, :],
                            vt[t][:, hB * HD1:(hB + 1) * HD1],
                            pt[:, SC:2 * SC],
                            start=(t == 0), stop=(t == NT - 1))
                    # Normalization. Sums rows go straight from PSUM to a
                    # DRAM bounce; V rows are evicted on DVE (frees the PSUM
                    # banks fast). Head B's rows DMA (unnormalized) to
                    # partitions 64-127 early, overlapping the reciprocal
                    # chain; the in-place multiply lands last. All bulk DMAs
                    # are split 2-way across queues.
                    rd = dram_pool.tile([2, SC], F32, name="rdtile", tag="rd")
                    ocA = rs_pool.tile([P, SC], F32, name="ocA", tag="ocA")
                    ocB = rs_pool.tile([P, SC], F32, name="ocB", tag="ocB")
                    nc.vector.tensor_copy(ocA[0:HD1, :], o_psA[0:HD1, :])
                    nc.vector.tensor_copy(ocB[0:HD1, :], o_psB[0:HD1, :])
                    nc.sync.dma_start(rd[0:1, :], ocA[HD:HD1, :])
                    nc.sync.dma_start(rd[1:2, :], ocB[HD:HD1, :])
                    HQ = HD // 2
                    nc.sync.dma_start(
                        on_tiles[hp][HD:HD + HQ, :].bitcast(F32), ocB[0:HQ, :])
                    nc.sync.dma_start(
                        on_tiles[hp][HD + HQ:P, :].bitcast(F32), ocB[HQ:HD, :])
                    # reciprocal of the two sums rows on all 128 DVE lanes:
                    # reload the DRAM rows as [64,8] partition-spread, recip
                    # once (bf16 out), bounce back, broadcast to [128, 512].
                    rsp = rs_pool.tile([P, SC // HD], F32, name="rsp", tag="rsp")
                    nc.sync.dma_start(
                        rsp[0:HD, :],
                        rd[0:1, :].rearrange("a (p c) -> (a p) c", c=SC // HD))
                    nc.sync.dma_start(
                        rsp[HD:P, :],
                        rd[1:2, :].rearrange("a (p c) -> (a p) c", c=SC // HD))
                    rspb = rs_pool.tile([P, SC // HD], BF16, name="rspb", tag="rspb")
                    with nc.allow_low_precision(reason="bf16 softmax recip broadcast"):
                        nc.vector.reciprocal(rspb[:], rsp[:])
                    rdb = dram_pool.tile([2, SC], BF16, name="rdbtile", tag="rdb")
                    nc.sync.dma_start(
                        rdb[0:1, :].rearrange("a (p c) -> (a p) c", c=SC // HD),
                        rspb[0:HD, :])
                    nc.sync.dma_start(
                        rdb[1:2, :].rearrange("a (p c) -> (a p) c", c=SC // HD),
                        rspb[HD:P, :])
                    # rb rows 0-63 = head-A recip, rows 64-127 = head-B recip
                    rb = rb_pool.tile([P, SC], BF16, name="rbtile", tag="rb")
                    nc.sync.dma_start(rb[0:HQ, :], rdb[0:1, :].to_broadcast((HQ, SC)))
                    nc.sync.dma_start(rb[HQ:HD, :], rdb[0:1, :].to_broadcast((HQ, SC)))
                    nc.sync.dma_start(rb[HD:HD + HQ, :], rdb[1:2, :].to_broadcast((HQ, SC)))
                    nc.sync.dma_start(rb[HD + HQ:P, :], rdb[1:2, :].to_broadcast((HQ, SC)))
                    nc.vector.tensor_mul(
                        on_tiles[hp][0:HD, :], ocA[0:HD, :], rb[0:HD, :])
                    nc.vector.tensor_mul(
                        on_tiles[hp][HD:P, :], on_tiles[hp][HD:P, :], rb[HD:P, :])
                # output projection for this s-chunk
                for dc in range(ND):
                    op_ps = ps_op.tile([P, SC], F32, name="opps", tag="op")
                    for e in range(NF):
                        nc.tensor.matmul(
                            op_ps[:], wo[e][:, dc * P:(dc + 1) * P], on_tiles[e][:],
                            start=(e == 0), stop=(e == NF - 1))
                    ot = ot_pool.tile([P, SC], F32, name="ottile", tag="ot")
                    nc.vector.tensor_scalar_add(ot[:], op_ps[:], bot[dc][:])
                    hc = SC // 2
                    nc.sync.dma_start(
                        outT[dc * P:(dc + 1) * P, s * SC:s * SC + hc], ot[:, 0:hc])
                    nc.sync.dma_start(
                        outT[dc * P:(dc + 1) * P, s * SC + hc:(s + 1) * SC], ot[:, hc:SC])

    nc.finalize()
    return nc


def _get_nc():
    if "nc" not in _NC_CACHE:
        _NC_CACHE["nc"] = _build_nc()
    return _NC_CACHE["nc"]


def _shard_inputs(x, w_qkv, b_qkv, w_out, b_out):
    """Build the 8 per-core input maps. Core i = (b = i//2, g = i%2)."""
    x = np.asarray(x, np.float32)
    w_qkv = np.asarray(w_qkv, np.float32)
    b_qkv = np.asarray(b_qkv, np.float32)
    w_out = np.asarray(w_out, np.float32)
    b_out = np.asarray(b_out, np.float32)

    BF = ml_dtypes.bfloat16
    in_maps = []
    for b in range(B):
        xT = np.ascontiguousarray(x[b].T.astype(BF))  # [D, S]
        for g in range(2):
            heads = range(g * HPG, (g + 1) * HPG)
            # w_qkv rows for head h: [192h, 192h+64) = Q, +64..128 = K, +128..192 = V
            q_rows = np.concatenate([np.arange(3 * HD * h, 3 * HD * h + HD) for h in heads])
            k_rows = q_rows + HD
            v_rows = q_rows + 2 * HD
            wqT = np.ascontiguousarray(w_qkv[q_rows].T.astype(BF))  # [D, E]
            wkT = np.ascontiguousarray(w_qkv[k_rows].T.astype(BF))
            wvT = np.ascontiguousarray(w_qkv[v_rows].T.astype(BF))
            ecols = np.arange(g * E, (g + 1) * E)
            woT = np.ascontiguousarray(w_out[:, ecols].T)  # [E, D]
            bo = b_out[:, None] if g == 0 else np.zeros((D, 1), np.float32)
            in_maps.append({
                "xT": xT,
                "wqT": wqT,
                "wkT": wkT,
                "wvT": wvT,
                "woT": woT,
                "bq": np.ascontiguousarray(b_qkv[q_rows][:, None]),
                "bk": np.ascontiguousarray(b_qkv[k_rows][:, None]),
                "bv": np.ascontiguousarray(b_qkv[v_rows][None, :]),
                "bo": np.ascontiguousarray(bo),
            })
    return in_maps


def run(inputs, trace=False):
    """Run the kernel; returns (full_output, exec_time_ns or None)."""
    nc = _get_nc()
    in_maps = _shard_inputs(**inputs)
    res = run_bass_kernel_spmd(nc, in_maps, core_ids=list(range(8)), trace=trace)
    out = np.empty((B, S, D), np.float32)
    for b in range(B):
        acc = res.results[2 * b]["outT"] + res.results[2 * b + 1]["outT"]
        out[b] = acc.T
    return out, res.exec_time_ns


def kernel(x, w_qkv, b_qkv, w_out, b_out):
    out, _ = run(dict(x=x, w_qkv=w_qkv, b_qkv=b_qkv, w_out=w_out, b_out=b_out))
    return out


# revision 17
# speedup vs baseline: 1.0371x; 1.0154x over previous
"""Multi-head self-attention (B=4, S=2048, D=1024, H=16) on 8 TRN2 NeuronCores.

Sharding: core i = (batch b = i//2, head-group g = i%2). Each core computes,
for its batch and its 8 heads: QKV projection, attention, and a partial
output projection over its 512 attention features. Host sums the two
partials per batch (Megatron-style tensor parallel over heads x data
parallel over batch).

V4: the attention inner loop is a single software pipeline flattened over
all (s-chunk, head-pair, t-tile) steps. Scores+exp for step k issue before
the PV matmuls of step k-1, so the PV never blocks the in-order PE queue
on the current exp, and the ScalarE exp stream (the 285us floor: 256 x
[128,1024] activations at ~1.1us) runs back to back with its input
dependency pre-satisfied. Q projections for s+1 and the output projection
for s-1 are deferred into specific steps of the pipeline to fill the PE
slack under the exp pacing. x is resident in SBUF (bf16) for the whole
kernel; V+K projections run as a short PE-bound prologue (Q(s0) too),
everything else lives inside the pipeline.

Numerics: Q/K/x/w_qkv in bf16 (scores matmul bf16; 1.7e-3 end-to-end),
V/P/out-proj in fp32r, softmax reciprocal broadcast in bf16, output
partials in bf16 (summed in fp32 on host). fp8 was evaluated and
rejected: attention output is a near-complete cancellation sum, so
per-element fp8 error (~4%) survives as ~3-5e-2 output error (gate 2e-2).

Per-core dataflow (transposed orientation so the softmax denominator
comes out of the PE array for free):
  V[t,e]   = x^T-stationary matmuls over Wv^T + ones column per head
  K^T[f,s] = Wk-stationary matmuls over x^T (bf16, full tensor resident)
  Q^T[f,s] = Wq-stationary, zero-padded per head half in a 2-deep ring
  S^T[t,s] = K^T-tile-stationary matmuls against qt2 halves
  P^T      = exp(S^T / 8) (ScalarE, PSUM->SBUF; fp32 exp needs no max-sub)
  O^T_aug  = V_aug-stationary matmuls over P^T (M=65); row 64 = denom
  On = O^T * recip(denom) (denoms spread via a DRAM bounce, one recip for
       both heads, bf16 broadcast); out^T = Wout^T-stationary over On.
"""
import os
import sys
import types

import ml_dtypes
import numpy as np

# ---------------------------------------------------------------------------
# environment bootstrap (self-contained: no problem-dir imports)
# ---------------------------------------------------------------------------


def _install_ntff_hook():
    """run_bass_kernel_spmd(trace=True) under axon needs antenv.axon_hooks,
    which the agent image's antenv stub lacks. Recreate it."""
    if "antenv.axon_hooks" in sys.modules:
        return
    try:
        import antenv
        from trn_agent_boot.trn_boot import _ntff_profile_via_ctypes
    except Exception:
        return
    so_path = "/opt/axon/libaxon_pjrt.so"
    if not os.path.exists(so_path):
        return
    mod = types.ModuleType("antenv.axon_hooks")
    _hook = [_ntff_profile_via_ctypes(so_path)]
    mod.get_axon_ntff_profile_hook = lambda: _hook[0]

    def _set(h):
        _hook[0] = h

    mod.set_axon_ntff_profile_hook = _set
    sys.modules["antenv.axon_hooks"] = mod
    antenv.axon_hooks = mod


_install_ntff_hook()

import concourse.bacc as bacc
import concourse.tile as tile
from concourse import mybir
from concourse.bass_utils import run_bass_kernel_spmd
from contextlib import ExitStack

# ---------------------------------------------------------------------------
# problem constants (hardcoded per contract)
# ---------------------------------------------------------------------------
B, S, D = 4, 2048, 1024
H, HD = 16, 64
HPG = 8            # heads per core (group)
E = HPG * HD       # 512 attention features per core
P = 128
SC = 512           # s-chunk
NS = S // SC       # 4 s-chunks
NT = S // P        # 16 t-chunks
ND = D // P        # 8 d-chunks
NF = E // P        # 4 f-chunks per Q (or K) = head-pairs
HD1 = HD + 1       # V_aug columns per head (V + ones)
HQ = HD // 2
QBLK = 2 * SC      # one s-block in the qt2 ring
SCALE = 1.0 / np.sqrt(np.float32(HD))

F32 = mybir.dt.float32
F32R = mybir.dt.float32r
BF16 = mybir.dt.bfloat16
EXP = mybir.ActivationFunctionType.Exp

_NC_CACHE = {}


def _build_nc():
    nc = bacc.Bacc("TRN2", target_bir_lowering=False)

    xT = nc.dram_tensor("xT", [D, S], BF16, kind="ExternalInput")
    wqT = nc.dram_tensor("wqT", [D, E], BF16, kind="ExternalInput")
    wkT = nc.dram_tensor("wkT", [D, E], BF16, kind="ExternalInput")
    wvT = nc.dram_tensor("wvT", [D, E], BF16, kind="ExternalInput")
    woT = nc.dram_tensor("woT", [E, D], F32R, kind="ExternalInput")
    bq = nc.dram_tensor("bq", [E, 1], F32, kind="ExternalInput")
    bk = nc.dram_tensor("bk", [E, 1], F32, kind="ExternalInput")
    bv = nc.dram_tensor("bv", [1, E], F32, kind="ExternalInput")
    bo = nc.dram_tensor("bo", [D, 1], F32, kind="ExternalInput")
    outT = nc.dram_tensor("outT", [D, S], BF16, kind="ExternalOutput")

    with tile.TileContext(nc) as tc, ExitStack() as glob:
        const = glob.enter_context(tc.tile_pool(name="const", bufs=1))
        bv_bc = const.tile([P, E], F32, name="bv_bc")
        resid = glob.enter_context(tc.tile_pool(name="resid", bufs=1))
        # qt2[f]: 2-deep ring of s-blocks, each block [half, SC]: half 0
        # holds head-A rows 0-63 (64-127 zero), half 1 the opposite.
        qt2 = [resid.tile([P, 2 * QBLK], BF16, name=f"qt2_{f}") for f in range(NF)]
        kt = [resid.tile([P, S], BF16, name=f"kt{f}") for f in range(NF)]
        vt = [resid.tile([P, HPG * HD1], F32R, name=f"vt{t}") for t in range(NT)]
        xfull = [resid.tile([P, S], BF16, name=f"xf{d}") for d in range(ND)]
        for f in range(NF):
            qv = qt2[f][:].rearrange("p (r h c) -> p r h c", h=2, c=SC)
            nc.vector.memset(qv[HD:P, :, 0, :], 0.0)
            nc.vector.memset(qv[0:HD, :, 1, :], 0.0)
        gw = glob.enter_context(tc.tile_pool(name="gw", bufs=1))
        wq = [gw.tile([P, E], BF16, name=f"wq{d}") for d in range(ND)]
        bqt = [gw.tile([P, 1], F32, name=f"bqt{f}") for f in range(NF)]

        # ---------------- phase 1: V + K (+ Q for s0) --------------------
        with ExitStack() as c1:
            wpool = c1.enter_context(tc.tile_pool(name="w", bufs=1))
            wv = [wpool.tile([P, E], BF16, name=f"wv{d}") for d in range(ND)]
            wk = [wpool.tile([P, E], BF16, name=f"wk{d}") for d in range(ND)]
            bkt = [wpool.tile([P, 1], F32, name=f"bkt{f}") for f in range(NF)]
            # DMA issue order: first-needed first
            for d in range(ND):
                nc.sync.dma_start(wv[d][:], wvT[d * P:(d + 1) * P, :])
            for d in range(ND):
                nc.sync.dma_start(xfull[d][:, 0:SC], xT[d * P:(d + 1) * P, 0:SC])
            for d in range(ND):
                nc.sync.dma_start(wk[d][:], wkT[d * P:(d + 1) * P, :])
            for s in range(1, NS):
                for d in range(ND):
                    nc.sync.dma_start(
                        xfull[d][:, s * SC:(s + 1) * SC],
                        xT[d * P:(d + 1) * P, s * SC:(s + 1) * SC])
            for d in range(ND):
                nc.sync.dma_start(wq[d][:], wqT[d * P:(d + 1) * P, :])
            nc.sync.dma_start(bv_bc[:], bv[0:1, :].to_broadcast((P, E)))
            for f in range(NF):
                nc.sync.dma_start(bqt[f][:], bq[f * P:(f + 1) * P, :])
                nc.sync.dma_start(bkt[f][:], bk[f * P:(f + 1) * P, :])

            psv = c1.enter_context(tc.tile_pool(name="psv", bufs=2, space="PSUM"))
            psq = c1.enter_context(tc.tile_pool(name="psq", bufs=4, space="PSUM"))

            for s in range(NS):
                sl = slice(s * SC, (s + 1) * SC)
                # V: x-stationary, stream Wv (out [t, 512 feats])
                for i in range(NS):
                    t = s * NS + i
                    ps = psv.tile([P, E], F32, name="psvt", tag="psv")
                    for d in range(ND):
                        nc.tensor.matmul(
                            ps[:], xfull[d][:, t * P:(t + 1) * P], wv[d][:],
                            start=(d == 0), stop=(d == ND - 1))
                    vdst = vt[t][:].rearrange("p (h c) -> p h c", c=HD1)
                    nc.vector.tensor_add(
                        vdst[:, :, 0:HD],
                        ps[:].rearrange("p (h c) -> p h c", c=HD),
                        bv_bc[:].rearrange("p (h c) -> p h c", c=HD))
                    nc.vector.memset(vdst[:, :, HD:HD1].bitcast(F32), 1.0)
                # K: Wk-stationary, stream x (out [kfeat, 512 s-cols])
                for f in range(NF):
                    ps = psq.tile([P, SC], F32, name="pskt", tag="psq")
                    for d in range(ND):
                        nc.tensor.matmul(
                            ps[:], wk[d][:, f * P:(f + 1) * P], xfull[d][:, sl],
                            start=(d == 0), stop=(d == ND - 1))
                    nc.vector.tensor_scalar_add(kt[f][:, sl], ps[:], bkt[f][:])
            # Q for s0 (later s-chunks stream inside the attention pipeline)
            for f in range(NF):
                ps = psq.tile([P, SC], F32, name="psqt", tag="psq")
                for d in range(ND):
                    nc.tensor.matmul(
                        ps[:], wq[d][:, f * P:(f + 1) * P], xfull[d][:, 0:SC],
                        start=(d == 0), stop=(d == ND - 1))
                nc.vector.tensor_scalar_add(
                    qt2[f][0:HD, 0:SC], ps[0:HD, :], bqt[f][0:HD, :])
                nc.vector.tensor_scalar_add(
                    qt2[f][HD:P, SC:QBLK], ps[HD:P, :], bqt[f][HD:P, :])

        # ---------------- phase 2: flattened attention pipeline ----------
        with ExitStack() as c2:
            wo_pool = c2.enter_context(tc.tile_pool(name="wo", bufs=1))
            wo = [wo_pool.tile([P, D], F32R, name=f"wo{e}") for e in range(NF)]
            for e in range(NF):
                nc.sync.dma_start(wo[e][:], woT[e * P:(e + 1) * P, :])
            bot = [wo_pool.tile([P, 1], F32, name=f"bot{i}") for i in range(ND)]
            for i in range(ND):
                nc.sync.dma_start(bot[i][:], bo[i * P:(i + 1) * P, :])

            dram_pool = c2.enter_context(tc.tile_pool(name="dramrs", bufs=3, space="DRAM"))
            pt_pool = c2.enter_context(tc.tile_pool(name="pt", bufs=5))
            on_pool = c2.enter_context(tc.tile_pool(name="on", bufs=2))
            rs_pool = c2.enter_context(tc.tile_pool(name="rs", bufs=3))
            rb_pool = c2.enter_context(tc.tile_pool(name="rb", bufs=3))
            ot_pool = c2.enter_context(tc.tile_pool(name="ot", bufs=4))
            ps_sc = c2.enter_context(tc.tile_pool(name="ps_sc", bufs=2, space="PSUM"))
            ps_o = c2.enter_context(tc.tile_pool(name="ps_o", bufs=1, space="PSUM"))
            ps_op = c2.enter_context(tc.tile_pool(name="ps_op", bufs=2, space="PSUM"))

            on_s = {}

            def alloc_on(s):
                if s not in on_s:
                    on_s[s] = [on_pool.tile([P, SC], F32R, name="on", tag=f"on{hp}")
                               for hp in range(NF)]
                return on_s[s]

            def emit_norm(s, hp, o_psA, o_psB):
                on_t = alloc_on(s)[hp]
                ocA = rs_pool.tile([P, SC], F32, name="ocA", tag="ocA")
                ocB = rs_pool.tile([P, SC], F32, name="ocB", tag="ocB")
                nc.vector.tensor_copy(ocA[0:HD1, :], o_psA[0:HD1, :])
                nc.vector.tensor_copy(ocB[0:HD1, :], o_psB[0:HD1, :])
                rd = dram_pool.tile([2, SC], F32, name="rdtile", tag="rd")
                nc.sync.dma_start(rd[0:1, :], ocA[HD:HD1, :])
                nc.sync.dma_start(rd[1:2, :], ocB[HD:HD1, :])
                # head B rows move (unnormalized) to partitions 64-127 early,
                # overlapping the reciprocal chain; normalized in place below.
                nc.sync.dma_start(
                    on_t[HD:HD + HQ, :].bitcast(F32), ocB[0:HQ, :])
                nc.sync.dma_start(
                    on_t[HD + HQ:P, :].bitcast(F32), ocB[HQ:HD, :])
                # both sums rows spread over 128 partitions in one DMA
                rsp = rs_pool.tile([P, SC // HD], F32, name="rsp", tag="rsp")
                nc.sync.dma_start(
                    rsp[:], rd[0:2, :].rearrange("a (p c) -> (a p) c", c=SC // HD))
                rspb = rs_pool.tile([P, SC // HD], BF16, name="rspb", tag="rspb")
                with nc.allow_low_precision(reason="bf16 softmax recip broadcast"):
                    nc.vector.reciprocal(rspb[:], rsp[:])
                rdb = dram_pool.tile([2, SC], BF16, name="rdbtile", tag="rdb")
                nc.sync.dma_start(
                    rdb[0:2, :].rearrange("a (p c) -> (a p) c", c=SC // HD),
                    rspb[:])
                rb = rb_pool.tile([P, SC], BF16, name="rbtile", tag="rb")
                nc.sync.dma_start(rb[0:HD, :], rdb[0:1, :].to_broadcast((HD, SC)))
                nc.sync.dma_start(rb[HD:P, :], rdb[1:2, :].to_broadcast((HD, SC)))
                nc.vector.tensor_mul(on_t[0:HD, :], ocA[0:HD, :], rb[0:HD, :])
                nc.vector.tensor_mul(on_t[HD:P, :], on_t[HD:P, :], rb[HD:P, :])

            def emit_q(sn, f):
                blk = (sn % 2) * QBLK
                sl = slice(sn * SC, (sn + 1) * SC)
                ps = ps_op.tile([P, SC], F32, name="psqd", tag="op")
                for d in range(ND):
                    nc.tensor.matmul(
                        ps[:], wq[d][:, f * P:(f + 1) * P], xfull[d][:, sl],
                        start=(d == 0), stop=(d == ND - 1))
                nc.vector.tensor_scalar_add(
                    qt2[f][0:HD, blk:blk + SC], ps[0:HD, :], bqt[f][0:HD, :])
                nc.vector.tensor_scalar_add(
                    qt2[f][HD:P, blk + SC:blk + QBLK], ps[HD:P, :], bqt[f][HD:P, :])

            def emit_outproj_dc(s, dc):
                on_tiles = on_s[s]
                op_ps = ps_op.tile([P, SC], F32, name="opps", tag="op")
                for e in range(NF):
                    nc.tensor.matmul(
                        op_ps[:], wo[e][:, dc * P:(dc + 1) * P], on_tiles[e][:],
                        start=(e == 0), stop=(e == NF - 1))
                ot = ot_pool.tile([P, SC], BF16, name="ottile", tag="ot")
                nc.vector.tensor_scalar_add(ot[:], op_ps[:], bot[dc][:])
                hc = SC // 2
                nc.sync.dma_start(
                    outT[dc * P:(dc + 1) * P, s * SC:s * SC + hc], ot[:, 0:hc])
                nc.sync.dma_start(
                    outT[dc * P:(dc + 1) * P, s * SC + hc:(s + 1) * SC], ot[:, hc:SC])

            cur_ps = [None]

            def emit_pv(s, hp, t, pt):
                if t == 0:
                    cur_ps[0] = (
                        ps_o.tile([P, SC], F32, name="opsA", tag="oA"),
                        ps_o.tile([P, SC], F32, name="opsB", tag="oB"))
                o_psA, o_psB = cur_ps[0]
                hA, hB = 2 * hp, 2 * hp + 1
                nc.tensor.matmul(
                    o_psA[0:HD1, :], vt[t][:, hA * HD1:(hA + 1) * HD1],
                    pt[:, 0:SC], start=(t == 0), stop=(t == NT - 1))
                nc.tensor.matmul(
                    o_psB[0:HD1, :], vt[t][:, hB * HD1:(hB + 1) * HD1],
                    pt[:, SC:2 * SC], start=(t == 0), stop=(t == NT - 1))
                if t == NT - 1:
                    emit_norm(s, hp, o_psA, o_psB)

            # deferred work: Q(s+1) mid-group; outproj(s) early in s+1
            due = {}
            def _idx(s, hp, t):
                return (s * NF + hp) * NT + t
            for s in range(NS - 1):
                for f in range(NF):
                    due.setdefault(_idx(s, f, 8), []).append(
                        (emit_q, (s + 1, f)))
                for dc in range(ND):
                    due.setdefault(_idx(s + 1, 0, 2) + 2 * dc, []).append(
                        (emit_outproj_dc, (s, dc)))

            pending = None
            for s in range(NS):
                blk = (s % 2) * QBLK
                for hp in range(NF):
                    for t in range(NT):
                        k = _idx(s, hp, t)
                        tsl = slice(t * P, (t + 1) * P)
                        sc_ps = ps_sc.tile([P, 2 * SC], F32, name="scps", tag="sc")
                        nc.tensor.matmul(
                            sc_ps[:, 0:SC], kt[hp][:, tsl],
                            qt2[hp][:, blk:blk + SC], start=True, stop=True)
                        nc.tensor.matmul(
                            sc_ps[:, SC:2 * SC], kt[hp][:, tsl],
                            qt2[hp][:, blk + SC:blk + QBLK], start=True, stop=True)
                        pt = pt_pool.tile([P, 2 * SC], F32R, name="ptile", tag="pt")
                        nc.scalar.activation(pt[:], sc_ps[:], EXP, scale=float(SCALE))
                        if pending is not None:
                            emit_pv(*pending)
                        pending = (s, hp, t, pt)
                        for fn, args in due.get(k, []):
                            fn(*args)
            emit_pv(*pending)
            for dc in range(ND):
                emit_outproj_dc(NS - 1, dc)

    nc.finalize()
    return nc


def _get_nc():
    if "nc" not in _NC_CACHE:
        _NC_CACHE["nc"] = _build_nc()
    return _NC_CACHE["nc"]


def _shard_inputs(x, w_qkv, b_qkv, w_out, b_out):
    """Build the 8 per-core input maps. Core i = (b = i//2, g = i%2)."""
    x = np.asarray(x, np.float32)
    w_qkv = np.asarray(w_qkv, np.float32)
    b_qkv = np.asarray(b_qkv, np.float32)
    w_out = np.asarray(w_out, np.float32)
    b_out = np.asarray(b_out, np.float32)

    BF = ml_dtypes.bfloat16
    in_maps = []
    for b in range(B):
        xT = np.ascontiguousarray(x[b].T.astype(BF))  # [D, S]
        for g in range(2):
            heads = range(g * HPG, (g + 1) * HPG)
            # w_qkv rows for head h: [192h, 192h+64) = Q, +64..128 = K, +128..192 = V
            q_rows = np.concatenate([np.arange(3 * HD * h, 3 * HD * h + HD) for h in heads])
            k_rows = q_rows + HD
            v_rows = q_rows + 2 * HD
            wqT = np.ascontiguousarray(w_qkv[q_rows].T.astype(BF))  # [D, E]
            wkT = np.ascontiguousarray(w_qkv[k_rows].T.astype(BF))
            wvT = np.ascontiguousarray(w_qkv[v_rows].T.astype(BF))
            ecols = np.arange(g * E, (g + 1) * E)
            woT = np.ascontiguousarray(w_out[:, ecols].T)  # [E, D]
            bo = b_out[:, None] if g == 0 else np.zeros((D, 1), np.float32)
            in_maps.append({
                "xT": xT,
                "wqT": wqT,
                "wkT": wkT,
                "wvT": wvT,
                "woT": woT,
                "bq": np.ascontiguousarray(b_qkv[q_rows][:, None]),
                "bk": np.ascontiguousarray(b_qkv[k_rows][:, None]),
                "bv": np.ascontiguousarray(b_qkv[v_rows][None, :]),
                "bo": np.ascontiguousarray(bo),
            })
    return in_maps


def run(inputs, trace=False):
    """Run the kernel; returns (full_output, exec_time_ns or None)."""
    nc = _get_nc()
    in_maps = _shard_inputs(**inputs)
    res = run_bass_kernel_spmd(nc, in_maps, core_ids=list(range(8)), trace=trace)
    out = np.empty((B, S, D), np.float32)
    for b in range(B):
        acc = (res.results[2 * b]["outT"].astype(np.float32)
               + res.results[2 * b + 1]["outT"].astype(np.float32))
        out[b] = acc.T
    return out, res.exec_time_ns


def kernel(x, w_qkv, b_qkv, w_out, b_out):
    out, _ = run(dict(x=x, w_qkv=w_qkv, b_qkv=b_qkv, w_out=w_out, b_out=b_out))
    return out


# revision 18
# speedup vs baseline: 1.1105x; 1.0708x over previous
"""Multi-head self-attention (B=4, S=2048, D=1024, H=16) on 8 TRN2 NeuronCores.

Sharding: core i = (batch b = i//2, head-group g = i%2). Each core computes,
for its batch and its 8 heads: QKV projection, attention, and a partial
output projection over its 512 attention features. Host sums the two
partials per batch (Megatron-style tensor parallel over heads x data
parallel over batch).

V5 notes (what matters on this hardware, measured):
  - ScalarE exp is the pacing engine: 256 x [128,1024] ACTIVATEs at
    ~1.11us + ~0.1us semaphore evaluation each = ~313us floor. Batching
    wider needs >8 PSUM banks. The attention loop is flattened over all
    (s, head-pair, t) steps with scores+exp issued one step ahead of the
    PV matmuls so the exp stream never waits on the in-order PE queue.
  - fp32 [128,128] stationary operands cannot double-buffer in the PE
    weight RAM, so their LDWEIGHTS serialize (~190ns each). Everything
    that loads stationary weights in the steady state (K^T tiles, V
    tiles, Wq, Wout) is bf16; P^T and On are bf16 so those matmuls are
    pure-bf16. PSUM accumulation stays fp32.
  - Every dma_start costs ~0.65us of serial sync-engine dispatch, so
    inputs are loaded with ONE dma_start per tensor (interleaved-AP
    scatter into a single wide SBUF tile), not per 128-row tile.
  - The softmax denominators bounce through DRAM to spread across
    partitions (engines cannot partition-broadcast); the chain is
    latency- not bandwidth-bound, so it is kept to 7 hops and head-B
    rows move to partitions 64-127 (unnormalized, bf16) in parallel.
  - fp8 was evaluated and rejected: attention output is a cancellation
    sum, so per-element fp8 error (~4%) survives as ~3-5e-2 output
    error (gate 2e-2). bf16 measures ~4-5e-3 end to end.

Per-core dataflow (transposed orientation so the softmax denominator
comes out of the PE array for free):
  V[t,e]   = x^T-stationary matmuls over Wv^T + ones column per head
  K^T[f,s] = Wk-stationary matmuls over x^T
  Q^T[f,s] = Wq-stationary, zero-padded per head half in a 2-deep ring
  S^T[t,s] = K^T-tile-stationary matmuls against qt2 halves
  P^T      = exp(S^T / 8) (ScalarE, PSUM->SBUF bf16; no max-sub needed)
  O^T_aug  = V_aug-stationary matmuls over P^T (M=65); row 64 = denom
  On = O^T * recip(denom); out^T = Wout^T-stationary over On (bf16
  partials, summed in fp32 on the host).
"""
import os
import sys
import types

import ml_dtypes
import numpy as np

# ---------------------------------------------------------------------------
# environment bootstrap (self-contained: no problem-dir imports)
# ---------------------------------------------------------------------------


def _install_ntff_hook():
    """run_bass_kernel_spmd(trace=True) under axon needs antenv.axon_hooks,
    which the agent image's antenv stub lacks. Recreate it."""
    if "antenv.axon_hooks" in sys.modules:
        return
    try:
        import antenv
        from trn_agent_boot.trn_boot import _ntff_profile_via_ctypes
    except Exception:
        return
    so_path = "/opt/axon/libaxon_pjrt.so"
    if not os.path.exists(so_path):
        return
    mod = types.ModuleType("antenv.axon_hooks")
    _hook = [_ntff_profile_via_ctypes(so_path)]
    mod.get_axon_ntff_profile_hook = lambda: _hook[0]

    def _set(h):
        _hook[0] = h

    mod.set_axon_ntff_profile_hook = _set
    sys.modules["antenv.axon_hooks"] = mod
    antenv.axon_hooks = mod


_install_ntff_hook()

import concourse.bacc as bacc
import concourse.tile as tile
from concourse import mybir
from concourse.bass_utils import run_bass_kernel_spmd
from contextlib import ExitStack

# ---------------------------------------------------------------------------
# problem constants (hardcoded per contract)
# ---------------------------------------------------------------------------
B, S, D = 4, 2048, 1024
H, HD = 16, 64
HPG = 8            # heads per core (group)
E = HPG * HD       # 512 attention features per core
P = 128
SC = 512           # s-chunk
NS = S // SC       # 4 s-chunks
NT = S // P        # 16 t-chunks
ND = D // P        # 8 d-chunks
NF = E // P        # 4 f-chunks per Q (or K) = head-pairs
HD1 = HD + 1       # V_aug columns per head (V + ones)
QBLK = 2 * SC      # one s-block in the qt2 ring
SCALE = 1.0 / np.sqrt(np.float32(HD))

F32 = mybir.dt.float32
F32R = mybir.dt.float32r
BF16 = mybir.dt.bfloat16
EXP = mybir.ActivationFunctionType.Exp

_NC_CACHE = {}


def _build_nc():
    nc = bacc.Bacc("TRN2", target_bir_lowering=False)

    xT = nc.dram_tensor("xT", [D, S], BF16, kind="ExternalInput")
    wqT = nc.dram_tensor("wqT", [D, E], BF16, kind="ExternalInput")
    wkT = nc.dram_tensor("wkT", [D, E], BF16, kind="ExternalInput")
    wvT = nc.dram_tensor("wvT", [D, E], BF16, kind="ExternalInput")
    woT = nc.dram_tensor("woT", [E, D], BF16, kind="ExternalInput")
    bq = nc.dram_tensor("bq", [E, 1], F32, kind="ExternalInput")
    bk = nc.dram_tensor("bk", [E, 1], F32, kind="ExternalInput")
    bv = nc.dram_tensor("bv", [1, E], F32, kind="ExternalInput")
    bo = nc.dram_tensor("bo", [D, 1], F32, kind="ExternalInput")
    outT = nc.dram_tensor("outT", [D, S], BF16, kind="ExternalOutput")

    with tile.TileContext(nc) as tc, ExitStack() as glob:
        const = glob.enter_context(tc.tile_pool(name="const", bufs=1))
        bv_bc = const.tile([P, E], F32, name="bv_bc")
        resid = glob.enter_context(tc.tile_pool(name="resid", bufs=1))
        # qt2[f]: 2-deep ring of s-blocks, each block [half, SC]: half 0
        # holds head-A rows 0-63 (64-127 zero), half 1 the opposite.
        qt2 = [resid.tile([P, 2 * QBLK], BF16, name=f"qt2_{f}") for f in range(NF)]
        kt = [resid.tile([P, S], BF16, name=f"kt{f}") for f in range(NF)]
        vt = [resid.tile([P, HPG * HD1], BF16, name=f"vt{t}") for t in range(NT)]
        xf = resid.tile([P, ND * S], BF16, name="xf")  # x^T, d-major blocks
        for f in range(NF):
            qv = qt2[f][:].rearrange("p (r h c) -> p r h c", h=2, c=SC)
            nc.vector.memset(qv[HD:P, :, 0, :], 0.0)
            nc.vector.memset(qv[0:HD, :, 1, :], 0.0)
        gw = glob.enter_context(tc.tile_pool(name="gw", bufs=1))
        wq = gw.tile([P, ND * E], BF16, name="wq")
        bqt = gw.tile([P, NF], F32, name="bqt")

        def xsl(d, lo, hi):
            return xf[:, d * S + lo:d * S + hi]

        # ---------------- phase 1: V + K (+ Q for s0) --------------------
        with ExitStack() as c1:
            wpool = c1.enter_context(tc.tile_pool(name="w", bufs=1))
            wv = wpool.tile([P, ND * E], BF16, name="wv")
            wk = wpool.tile([P, ND * E], BF16, name="wk")
            bkt = wpool.tile([P, NF], F32, name="bkt")
            # one dma_start per tensor (sync-engine dispatch is ~0.65us
            # each, serial); first-needed first.
            nc.sync.dma_start(
                wv[:].rearrange("p (d e) -> p d e", e=E),
                wvT[:].rearrange("(d p) e -> p d e", p=P))
            nc.sync.dma_start(
                xf[:].rearrange("p (d c) -> p d c", c=S)[:, :, 0:SC],
                xT[:, 0:SC].rearrange("(d p) c -> p d c", p=P))
            nc.sync.dma_start(
                wk[:].rearrange("p (d e) -> p d e", e=E),
                wkT[:].rearrange("(d p) e -> p d e", p=P))
            for s in range(1, NS):
                nc.sync.dma_start(
                    xf[:].rearrange("p (d c) -> p d c", c=S)[:, :, s * SC:(s + 1) * SC],
                    xT[:, s * SC:(s + 1) * SC].rearrange("(d p) c -> p d c", p=P))
            nc.sync.dma_start(
                wq[:].rearrange("p (d e) -> p d e", e=E),
                wqT[:].rearrange("(d p) e -> p d e", p=P))
            nc.sync.dma_start(bv_bc[:], bv[0:1, :].to_broadcast((P, E)))
            nc.sync.dma_start(bqt[:], bq[:].rearrange("(f p) a -> p (f a)", p=P))
            nc.sync.dma_start(bkt[:], bk[:].rearrange("(f p) a -> p (f a)", p=P))

            psv = c1.enter_context(tc.tile_pool(name="psv", bufs=2, space="PSUM"))
            psq = c1.enter_context(tc.tile_pool(name="psq", bufs=4, space="PSUM"))

            for s in range(NS):
                sl = slice(s * SC, (s + 1) * SC)
                # V: x-stationary, stream Wv (out [t, 512 feats])
                for i in range(NS):
                    t = s * NS + i
                    ps = psv.tile([P, E], F32, name="psvt", tag="psv")
                    for d in range(ND):
                        nc.tensor.matmul(
                            ps[:], xsl(d, t * P, (t + 1) * P),
                            wv[:, d * E:(d + 1) * E],
                            start=(d == 0), stop=(d == ND - 1))
                    vdst = vt[t][:].rearrange("p (h c) -> p h c", c=HD1)
                    nc.vector.tensor_add(
                        vdst[:, :, 0:HD],
                        ps[:].rearrange("p (h c) -> p h c", c=HD),
                        bv_bc[:].rearrange("p (h c) -> p h c", c=HD))
                    nc.vector.memset(vdst[:, :, HD:HD1], 1.0)
                # K: Wk-stationary, stream x (out [kfeat, 512 s-cols])
                for f in range(NF):
                    ps = psq.tile([P, SC], F32, name="pskt", tag="psq")
                    for d in range(ND):
                        nc.tensor.matmul(
                            ps[:], wk[:, d * E + f * P:d * E + (f + 1) * P],
                            xsl(d, s * SC, (s + 1) * SC),
                            start=(d == 0), stop=(d == ND - 1))
                    nc.vector.tensor_scalar_add(
                        kt[f][:, sl], ps[:], bkt[:, f:f + 1])
            # Q for s0 (later s-chunks stream inside the attention pipeline)
            for f in range(NF):
                ps = psq.tile([P, SC], F32, name="psqt", tag="psq")
                for d in range(ND):
                    nc.tensor.matmul(
                        ps[:], wq[:, d * E + f * P:d * E + (f + 1) * P],
                        xsl(d, 0, SC), start=(d == 0), stop=(d == ND - 1))
                nc.vector.tensor_scalar_add(
                    qt2[f][0:HD, 0:SC], ps[0:HD, :], bqt[0:HD, f:f + 1])
                nc.vector.tensor_scalar_add(
                    qt2[f][HD:P, SC:QBLK], ps[HD:P, :], bqt[HD:P, f:f + 1])

        # ---------------- phase 2: flattened attention pipeline ----------
        with ExitStack() as c2:
            wo_pool = c2.enter_context(tc.tile_pool(name="wo", bufs=1))
            wo = wo_pool.tile([P, NF * D], BF16, name="wo")
            nc.sync.dma_start(
                wo[:].rearrange("p (e c) -> p e c", c=D),
                woT[:].rearrange("(e p) c -> p e c", p=P))
            bot = wo_pool.tile([P, ND], F32, name="bot")
            nc.sync.dma_start(bot[:], bo[:].rearrange("(i p) a -> p (i a)", p=P))

            dram_pool = c2.enter_context(tc.tile_pool(name="dramrs", bufs=3, space="DRAM"))
            pt_pool = c2.enter_context(tc.tile_pool(name="pt", bufs=8))
            on_pool = c2.enter_context(tc.tile_pool(name="on", bufs=2))
            rs_pool = c2.enter_context(tc.tile_pool(name="rs", bufs=3))
            rb_pool = c2.enter_context(tc.tile_pool(name="rb", bufs=3))
            ot_pool = c2.enter_context(tc.tile_pool(name="ot", bufs=4))
            ps_sc = c2.enter_context(tc.tile_pool(name="ps_sc", bufs=2, space="PSUM"))
            ps_o = c2.enter_context(tc.tile_pool(name="ps_o", bufs=1, space="PSUM"))
            ps_op = c2.enter_context(tc.tile_pool(name="ps_op", bufs=2, space="PSUM"))

            on_s = {}

            def alloc_on(s):
                if s not in on_s:
                    on_s[s] = [on_pool.tile([P, SC], BF16, name="on", tag=f"on{hp}")
                               for hp in range(NF)]
                return on_s[s]

            def emit_norm(s, hp, o_psA, o_psB):
                on_t = alloc_on(s)[hp]
                ocA = rs_pool.tile([P, SC], F32, name="ocA", tag="ocA")
                ocB = rs_pool.tile([P, SC], BF16, name="ocB", tag="ocB")
                sB = rs_pool.tile([P, SC], F32, name="sB", tag="sB")
                nc.vector.tensor_copy(ocA[0:HD1, :], o_psA[0:HD1, :])
                nc.vector.tensor_copy(ocB[0:HD, :], o_psB[0:HD, :])
                nc.vector.tensor_copy(sB[HD:HD1, :], o_psB[HD:HD1, :])
                rd = dram_pool.tile([2, SC], F32, name="rdtile", tag="rd")
                nc.sync.dma_start(rd[0:1, :], ocA[HD:HD1, :])
                nc.sync.dma_start(rd[1:2, :], sB[HD:HD1, :])
                # head B rows move (unnormalized) to partitions 64-127,
                # overlapping the reciprocal chain; normalized in place.
                nc.sync.dma_start(on_t[HD:P, :], ocB[0:HD, :])
                # both sums rows spread over 128 partitions in one DMA
                rsp = rs_pool.tile([P, SC // HD], F32, name="rsp", tag="rsp")
                nc.sync.dma_start(
                    rsp[:], rd[0:2, :].rearrange("a (p c) -> (a p) c", c=SC // HD))
                rspb = rs_pool.tile([P, SC // HD], BF16, name="rspb", tag="rspb")
                with nc.allow_low_precision(reason="bf16 softmax recip broadcast"):
                    nc.vector.reciprocal(rspb[:], rsp[:])
                rdb = dram_pool.tile([2, SC], BF16, name="rdbtile", tag="rdb")
                nc.sync.dma_start(
                    rdb[0:2, :].rearrange("a (p c) -> (a p) c", c=SC // HD),
                    rspb[:])
                rb = rb_pool.tile([P, SC], BF16, name="rbtile", tag="rb")
                nc.sync.dma_start(rb[0:HD, :], rdb[0:1, :].to_broadcast((HD, SC)))
                nc.sync.dma_start(rb[HD:P, :], rdb[1:2, :].to_broadcast((HD, SC)))
                nc.vector.tensor_mul(on_t[0:HD, :], ocA[0:HD, :], rb[0:HD, :])
                nc.vector.tensor_mul(on_t[HD:P, :], on_t[HD:P, :], rb[HD:P, :])

            def emit_q(sn, f):
                blk = (sn % 2) * QBLK
                ps = ps_op.tile([P, SC], F32, name="psqd", tag="op")
                for d in range(ND):
                    nc.tensor.matmul(
                        ps[:], wq[:, d * E + f * P:d * E + (f + 1) * P],
                        xsl(d, sn * SC, (sn + 1) * SC),
                        start=(d == 0), stop=(d == ND - 1))
                nc.vector.tensor_scalar_add(
                    qt2[f][0:HD, blk:blk + SC], ps[0:HD, :], bqt[0:HD, f:f + 1])
                nc.vector.tensor_scalar_add(
                    qt2[f][HD:P, blk + SC:blk + QBLK], ps[HD:P, :], bqt[HD:P, f:f + 1])

            def emit_outproj_dc(s, dc):
                on_tiles = on_s[s]
                op_ps = ps_op.tile([P, SC], F32, name="opps", tag="op")
                for e in range(NF):
                    nc.tensor.matmul(
                        op_ps[:], wo[:, e * D + dc * P:e * D + (dc + 1) * P],
                        on_tiles[e][:], start=(e == 0), stop=(e == NF - 1))
                ot = ot_pool.tile([P, SC], BF16, name="ottile", tag="ot")
                nc.vector.tensor_scalar_add(ot[:], op_ps[:], bot[:, dc:dc + 1])
                nc.sync.dma_start(
                    outT[dc * P:(dc + 1) * P, s * SC:(s + 1) * SC], ot[:])

            cur_ps = [None]

            def emit_pv(s, hp, t, pt):
                if t == 0:
                    cur_ps[0] = (
                        ps_o.tile([P, SC], F32, name="opsA", tag="oA"),
                        ps_o.tile([P, SC], F32, name="opsB", tag="oB"))
                o_psA, o_psB = cur_ps[0]
                hA, hB = 2 * hp, 2 * hp + 1
                nc.tensor.matmul(
                    o_psA[0:HD1, :], vt[t][:, hA * HD1:(hA + 1) * HD1],
                    pt[:, 0:SC], start=(t == 0), stop=(t == NT - 1))
                nc.tensor.matmul(
                    o_psB[0:HD1, :], vt[t][:, hB * HD1:(hB + 1) * HD1],
                    pt[:, SC:2 * SC], start=(t == 0), stop=(t == NT - 1))
                if t == NT - 1:
                    emit_norm(s, hp, o_psA, o_psB)

            # deferred work: Q(s+1) mid-group; outproj(s) early in s+1
            due = {}

            def _idx(s, hp, t):
                return (s * NF + hp) * NT + t

            for s in range(NS - 1):
                for f in range(NF):
                    due.setdefault(_idx(s, f, 8), []).append(
                        (emit_q, (s + 1, f)))
                for dc in range(ND):
                    due.setdefault(_idx(s + 1, 0, 2) + 2 * dc, []).append(
                        (emit_outproj_dc, (s, dc)))

            pending = None
            for s in range(NS):
                blk = (s % 2) * QBLK
                for hp in range(NF):
                    for t in range(NT):
                        k = _idx(s, hp, t)
                        tsl = slice(t * P, (t + 1) * P)
                        sc_ps = ps_sc.tile([P, 2 * SC], F32, name="scps", tag="sc")
                        nc.tensor.matmul(
                            sc_ps[:, 0:SC], kt[hp][:, tsl],
                            qt2[hp][:, blk:blk + SC], start=True, stop=True)
                        nc.tensor.matmul(
                            sc_ps[:, SC:2 * SC], kt[hp][:, tsl],
                            qt2[hp][:, blk + SC:blk + QBLK], start=True, stop=True)
                        pt = pt_pool.tile([P, 2 * SC], BF16, name="ptile", tag="pt")
                        nc.scalar.activation(pt[:], sc_ps[:], EXP, scale=float(SCALE))
                        if pending is not None:
                            emit_pv(*pending)
                        pending = (s, hp, t, pt)
                        for fn, args in due.get(k, []):
                            fn(*args)
            emit_pv(*pending)
            for dc in range(ND):
                emit_outproj_dc(NS - 1, dc)

    nc.finalize()
    return nc


def _get_nc():
    if "nc" not in _NC_CACHE:
        _NC_CACHE["nc"] = _build_nc()
    return _NC_CACHE["nc"]


def _shard_inputs(x, w_qkv, b_qkv, w_out, b_out):
    """Build the 8 per-core input maps. Core i = (b = i//2, g = i%2)."""
    x = np.asarray(x, np.float32)
    w_qkv = np.asarray(w_qkv, np.float32)
    b_qkv = np.asarray(b_qkv, np.float32)
    w_out = np.asarray(w_out, np.float32)
    b_out = np.asarray(b_out, np.float32)

    BF = ml_dtypes.bfloat16
    in_maps = []
    for b in range(B):
        xT = np.ascontiguousarray(x[b].T.astype(BF))  # [D, S]
        for g in range(2):
            heads = range(g * HPG, (g + 1) * HPG)
            # w_qkv rows for head h: [192h, 192h+64) = Q, +64..128 = K, +128..192 = V
            q_rows = np.concatenate([np.arange(3 * HD * h, 3 * HD * h + HD) for h in heads])
            k_rows = q_rows + HD
            v_rows = q_rows + 2 * HD
            wqT = np.ascontiguousarray(w_qkv[q_rows].T.astype(BF))  # [D, E]
            wkT = np.ascontiguousarray(w_qkv[k_rows].T.astype(BF))
            wvT = np.ascontiguousarray(w_qkv[v_rows].T.astype(BF))
            ecols = np.arange(g * E, (g + 1) * E)
            woT = np.ascontiguousarray(w_out[:, ecols].T.astype(BF))  # [E, D]
            bo = b_out[:, None] if g == 0 else np.zeros((D, 1), np.float32)
            in_maps.append({
                "xT": xT,
                "wqT": wqT,
                "wkT": wkT,
                "wvT": wvT,
                "woT": woT,
                "bq": np.ascontiguousarray(b_qkv[q_rows][:, None]),
                "bk": np.ascontiguousarray(b_qkv[k_rows][:, None]),
                "bv": np.ascontiguousarray(b_qkv[v_rows][None, :]),
                "bo": np.ascontiguousarray(bo),
            })
    return in_maps


def run(inputs, trace=False):
    """Run the kernel; returns (full_output, exec_time_ns or None)."""
    nc = _get_nc()
    in_maps = _shard_inputs(**inputs)
    res = run_bass_kernel_spmd(nc, in_maps, core_ids=list(range(8)), trace=trace)
    out = np.empty((B, S, D), np.float32)
    for b in range(B):
        acc = (res.results[2 * b]["outT"].astype(np.float32)
               + res.results[2 * b + 1]["outT"].astype(np.float32))
        out[b] = acc.T
    return out, res.exec_time_ns


def kernel(x, w_qkv, b_qkv, w_out, b_out):
    out, _ = run(dict(x=x, w_qkv=w_qkv, b_qkv=b_qkv, w_out=w_out, b_out=b_out))
    return out


# revision 28
# speedup vs baseline: 1.1557x; 1.0407x over previous
"""Multi-head self-attention (B=4, S=2048, D=1024, H=16) on 8 TRN2 NeuronCores.

Sharding: core i = (batch b = i//2, head-group g = i%2). Each core computes,
for its batch and its 8 heads: QKV projection, attention, and a partial
output projection over its 512 attention features. Host sums the two
partials per batch (Megatron-style tensor parallel over heads x data
parallel over batch).

V5 notes (what matters on this hardware, measured):
  - ScalarE exp is the pacing engine: 256 x [128,1024] ACTIVATEs at
    ~1.11us + ~0.1us semaphore evaluation each = ~313us floor. Batching
    wider needs >8 PSUM banks. The attention loop is flattened over all
    (s, head-pair, t) steps with scores+exp issued one step ahead of the
    PV matmuls so the exp stream never waits on the in-order PE queue.
  - fp32 [128,128] stationary operands cannot double-buffer in the PE
    weight RAM, so their LDWEIGHTS serialize (~190ns each). Everything
    that loads stationary weights in the steady state (K^T tiles, V
    tiles, Wq, Wout) is bf16; P^T and On are bf16 so those matmuls are
    pure-bf16. PSUM accumulation stays fp32.
  - Every dma_start costs ~0.65us of serial sync-engine dispatch, so
    inputs are loaded with ONE dma_start per tensor (interleaved-AP
    scatter into a single wide SBUF tile), not per 128-row tile.
  - The softmax denominators bounce through DRAM to spread across
    partitions (engines cannot partition-broadcast); the chain is
    latency- not bandwidth-bound, so it is kept to 7 hops and head-B
    rows move to partitions 64-127 (unnormalized, bf16) in parallel.
  - fp8 was evaluated and rejected: attention output is a cancellation
    sum, so per-element fp8 error (~4%) survives as ~3-5e-2 output
    error (gate 2e-2). bf16 measures ~4-5e-3 end to end.

Per-core dataflow (transposed orientation so the softmax denominator
comes out of the PE array for free):
  V[t,e]   = x^T-stationary matmuls over Wv^T + ones column per head
  K^T[f,s] = Wk-stationary matmuls over x^T
  Q^T[f,s] = Wq-stationary, zero-padded per head half in a 2-deep ring
  S^T[t,s] = K^T-tile-stationary matmuls against qt2 halves
  P^T      = exp(S^T / 8) (ScalarE, PSUM->SBUF bf16; no max-sub needed)
  O^T_aug  = V_aug-stationary matmuls over P^T (M=65); row 64 = denom
  On = O^T * recip(denom); out^T = Wout^T-stationary over On (bf16
  partials, summed in fp32 on the host).
"""
import os
import sys
import types

import ml_dtypes
import numpy as np

# ---------------------------------------------------------------------------
# environment bootstrap (self-contained: no problem-dir imports)
# ---------------------------------------------------------------------------


def _install_ntff_hook():
    """run_bass_kernel_spmd(trace=True) under axon needs antenv.axon_hooks,
    which the agent image's antenv stub lacks. Recreate it."""
    if "antenv.axon_hooks" in sys.modules:
        return
    try:
        import antenv
        from trn_agent_boot.trn_boot import _ntff_profile_via_ctypes
    except Exception:
        return
    so_path = "/opt/axon/libaxon_pjrt.so"
    if not os.path.exists(so_path):
        return
    mod = types.ModuleType("antenv.axon_hooks")
    _hook = [_ntff_profile_via_ctypes(so_path)]
    mod.get_axon_ntff_profile_hook = lambda: _hook[0]

    def _set(h):
        _hook[0] = h

    mod.set_axon_ntff_profile_hook = _set
    sys.modules["antenv.axon_hooks"] = mod
    antenv.axon_hooks = mod


_install_ntff_hook()

import concourse.bacc as bacc
import concourse.tile as tile
from concourse import mybir
from concourse.bass_utils import run_bass_kernel_spmd
from contextlib import ExitStack

# ---------------------------------------------------------------------------
# problem constants (hardcoded per contract)
# ---------------------------------------------------------------------------
B, S, D = 4, 2048, 1024
H, HD = 16, 64
HPG = 8            # heads per core (group)
E = HPG * HD       # 512 attention features per core
P = 128
SC = 512           # s-chunk
NS = S // SC       # 4 s-chunks
NT = S // P        # 16 t-chunks
ND = D // P        # 8 d-chunks
NF = E // P        # 4 f-chunks per Q (or K) = head-pairs
HD1 = HD + 1       # V_aug columns per head (V + ones)
QBLK = 2 * SC      # one s-block in the qt2 ring
SCALE = 1.0 / np.sqrt(np.float32(HD))

F32 = mybir.dt.float32
F32R = mybir.dt.float32r
BF16 = mybir.dt.bfloat16
EXP = mybir.ActivationFunctionType.Exp

_NC_CACHE = {}


def _build_nc():
    nc = bacc.Bacc("TRN2", target_bir_lowering=False)

    xT = nc.dram_tensor("xT", [D, S], BF16, kind="ExternalInput")
    wqT = nc.dram_tensor("wqT", [D, E], BF16, kind="ExternalInput")
    wkT = nc.dram_tensor("wkT", [D, E], BF16, kind="ExternalInput")
    wvT = nc.dram_tensor("wvT", [D, E], BF16, kind="ExternalInput")
    woT = nc.dram_tensor("woT", [E, D], BF16, kind="ExternalInput")
    bq = nc.dram_tensor("bq", [E, 1], F32, kind="ExternalInput")
    bk = nc.dram_tensor("bk", [E, 1], F32, kind="ExternalInput")
    bv = nc.dram_tensor("bv", [1, E], F32, kind="ExternalInput")
    bo = nc.dram_tensor("bo", [D, 1], F32, kind="ExternalInput")
    outT = nc.dram_tensor("outT", [D, S], BF16, kind="ExternalOutput")

    with tile.TileContext(nc) as tc, ExitStack() as glob:
        const = glob.enter_context(tc.tile_pool(name="const", bufs=1))
        bv_bc = const.tile([P, E], F32, name="bv_bc")
        resid = glob.enter_context(tc.tile_pool(name="resid", bufs=1))
        # qt2[f]: 2-deep ring of s-blocks, each block [half, SC]: half 0
        # holds head-A rows 0-63 (64-127 zero), half 1 the opposite.
        qt2 = [resid.tile([P, 2 * QBLK], BF16, name=f"qt2_{f}") for f in range(NF)]
        kt = [resid.tile([P, S], BF16, name=f"kt{f}") for f in range(NF)]
        vt = [resid.tile([P, HPG * HD1], BF16, name=f"vt{t}") for t in range(NT)]
        xf = resid.tile([P, ND * S], BF16, name="xf")  # x^T, d-major blocks
        for f in range(NF):
            qv = qt2[f][:].rearrange("p (r h c) -> p r h c", h=2, c=SC)
            nc.vector.memset(qv[HD:P, :, 0, :], 0.0)
            nc.vector.memset(qv[0:HD, :, 1, :], 0.0)
        gw = glob.enter_context(tc.tile_pool(name="gw", bufs=1))
        wq = gw.tile([P, ND * E], BF16, name="wq")
        wk = gw.tile([P, ND * E], BF16, name="wk")
        bqt = gw.tile([P, NF], F32, name="bqt")
        bkt = gw.tile([P, NF], F32, name="bkt")

        def xsl(d, lo, hi):
            return xf[:, d * S + lo:d * S + hi]

        # ---------------- phase 1: V + K (+ Q for s0) --------------------
        with ExitStack() as c1:
            wpool = c1.enter_context(tc.tile_pool(name="w", bufs=1))
            wv = wpool.tile([P, ND * E], BF16, name="wv")
            # one dma_start per tensor (sync-engine dispatch is ~0.65us
            # each, serial); first-needed first.
            hd2 = ND // 2
            for h in range(2):
                dsl = slice(h * hd2, (h + 1) * hd2)
                rsl = slice(h * hd2 * P, (h + 1) * hd2 * P)
                nc.sync.dma_start(
                    wv[:].rearrange("p (d e) -> p d e", e=E)[:, dsl, :],
                    wvT[rsl, :].rearrange("(d p) e -> p d e", p=P))
                nc.sync.dma_start(
                    xf[:].rearrange("p (d c) -> p d c", c=S)[:, dsl, 0:SC],
                    xT[rsl, 0:SC].rearrange("(d p) c -> p d c", p=P))
            nc.sync.dma_start(
                wk[:].rearrange("p (d e) -> p d e", e=E),
                wkT[:].rearrange("(d p) e -> p d e", p=P))
            for s in range(1, NS):
                nc.sync.dma_start(
                    xf[:].rearrange("p (d c) -> p d c", c=S)[:, :, s * SC:(s + 1) * SC],
                    xT[:, s * SC:(s + 1) * SC].rearrange("(d p) c -> p d c", p=P))
            nc.sync.dma_start(
                wq[:].rearrange("p (d e) -> p d e", e=E),
                wqT[:].rearrange("(d p) e -> p d e", p=P))
            nc.sync.dma_start(bv_bc[:], bv[0:1, :].to_broadcast((P, E)))
            nc.sync.dma_start(bqt[:], bq[:].rearrange("(f p) a -> p (f a)", p=P))
            nc.sync.dma_start(bkt[:], bk[:].rearrange("(f p) a -> p (f a)", p=P))

            psv = c1.enter_context(tc.tile_pool(name="psv", bufs=2, space="PSUM"))
            psq = c1.enter_context(tc.tile_pool(name="psq", bufs=4, space="PSUM"))

            for s in range(NS):
                # V: x-stationary, stream Wv (out [t, 512 feats])
                for i in range(NS):
                    t = s * NS + i
                    ps = psv.tile([P, E], F32, name="psvt", tag="psv")
                    for d in range(ND):
                        nc.tensor.matmul(
                            ps[:], xsl(d, t * P, (t + 1) * P),
                            wv[:, d * E:(d + 1) * E],
                            start=(d == 0), stop=(d == ND - 1))
                    vdst = vt[t][:].rearrange("p (h c) -> p h c", c=HD1)
                    nc.vector.tensor_add(
                        vdst[:, :, 0:HD],
                        ps[:].rearrange("p (h c) -> p h c", c=HD),
                        bv_bc[:].rearrange("p (h c) -> p h c", c=HD))
                    nc.vector.memset(vdst[:, :, HD:HD1], 1.0)
                # K for f=0 only; f=1..3 stream inside the attention pipeline
                ps = psq.tile([P, SC], F32, name="pskt", tag="psq")
                for d in range(ND):
                    nc.tensor.matmul(
                        ps[:], wk[:, d * E:d * E + P],
                        xsl(d, s * SC, (s + 1) * SC),
                        start=(d == 0), stop=(d == ND - 1))
                nc.vector.tensor_scalar_add(
                    kt[0][:, s * SC:(s + 1) * SC], ps[:], bkt[:, 0:1])
            # Q for s0 (later s-chunks stream inside the attention pipeline)
            for f in range(NF):
                ps = psq.tile([P, SC], F32, name="psqt", tag="psq")
                for d in range(ND):
                    nc.tensor.matmul(
                        ps[:], wq[:, d * E + f * P:d * E + (f + 1) * P],
                        xsl(d, 0, SC), start=(d == 0), stop=(d == ND - 1))
                nc.vector.tensor_scalar_add(
                    qt2[f][0:HD, 0:SC], ps[0:HD, :], bqt[0:HD, f:f + 1])
                nc.vector.tensor_scalar_add(
                    qt2[f][HD:P, SC:QBLK], ps[HD:P, :], bqt[HD:P, f:f + 1])

        # ---------------- phase 2: flattened attention pipeline ----------
        with ExitStack() as c2:
            wo_pool = c2.enter_context(tc.tile_pool(name="wo", bufs=1))
            wo = wo_pool.tile([P, NF * D], BF16, name="wo")
            nc.sync.dma_start(
                wo[:].rearrange("p (e c) -> p e c", c=D),
                woT[:].rearrange("(e p) c -> p e c", p=P))
            bot = wo_pool.tile([P, ND], F32, name="bot")
            nc.sync.dma_start(bot[:], bo[:].rearrange("(i p) a -> p (i a)", p=P))

            dram_pool = c2.enter_context(tc.tile_pool(name="dramrs", bufs=3, space="DRAM"))
            pt_pool = c2.enter_context(tc.tile_pool(name="pt", bufs=8))
            on_pool = c2.enter_context(tc.tile_pool(name="on", bufs=2))
            rs_pool = c2.enter_context(tc.tile_pool(name="rs", bufs=3))
            rb_pool = c2.enter_context(tc.tile_pool(name="rb", bufs=3))
            ot_pool = c2.enter_context(tc.tile_pool(name="ot", bufs=4))
            ps_sc = c2.enter_context(tc.tile_pool(name="ps_sc", bufs=2, space="PSUM"))
            ps_o = c2.enter_context(tc.tile_pool(name="ps_o", bufs=1, space="PSUM"))
            ps_op = c2.enter_context(tc.tile_pool(name="ps_op", bufs=2, space="PSUM"))

            on_s = {}

            def alloc_on(s):
                if s not in on_s:
                    on_s[s] = [on_pool.tile([P, SC], BF16, name="on", tag=f"on{hp}")
                               for hp in range(NF)]
                return on_s[s]

            def emit_norm(s, hp, o_psA, o_psB):
                on_t = alloc_on(s)[hp]
                # tiny sums-row copies first so the reciprocal DMA chain
                # launches before the bulk evictions finish
                sA = rs_pool.tile([P, SC], F32, name="sA", tag="sA")
                sB = rs_pool.tile([P, SC], F32, name="sB", tag="sB")
                nc.vector.tensor_copy(sA[HD:HD1, :], o_psA[HD:HD1, :])
                nc.vector.tensor_copy(sB[HD:HD1, :], o_psB[HD:HD1, :])
                rd = dram_pool.tile([2, SC], F32, name="rdtile", tag="rd")
                nc.sync.dma_start(rd[0:1, :], sA[HD:HD1, :])
                nc.sync.dma_start(rd[1:2, :], sB[HD:HD1, :])
                ocA = rs_pool.tile([P, SC], F32, name="ocA", tag="ocA")
                ocB = rs_pool.tile([P, SC], BF16, name="ocB", tag="ocB")
                nc.vector.tensor_copy(ocA[0:HD, :], o_psA[0:HD, :])
                nc.vector.tensor_copy(ocB[0:HD, :], o_psB[0:HD, :])
                # head B rows move (unnormalized) to partitions 64-127,
                # overlapping the reciprocal chain; normalized in place.
                nc.sync.dma_start(on_t[HD:P, :], ocB[0:HD, :])
                # both sums rows spread over 128 partitions in one DMA
                rsp = rs_pool.tile([P, SC // HD], F32, name="rsp", tag="rsp")
                nc.sync.dma_start(
                    rsp[:], rd[0:2, :].rearrange("a (p c) -> (a p) c", c=SC // HD))
                rspb = rs_pool.tile([P, SC // HD], BF16, name="rspb", tag="rspb")
                with nc.allow_low_precision(reason="bf16 softmax recip broadcast"):
                    nc.vector.reciprocal(rspb[:], rsp[:])
                rdb = dram_pool.tile([2, SC], BF16, name="rdbtile", tag="rdb")
                nc.sync.dma_start(
                    rdb[0:2, :].rearrange("a (p c) -> (a p) c", c=SC // HD),
                    rspb[:])
                rb = rb_pool.tile([P, SC], BF16, name="rbtile", tag="rb")
                nc.sync.dma_start(rb[0:HD, :], rdb[0:1, :].to_broadcast((HD, SC)))
                nc.sync.dma_start(rb[HD:P, :], rdb[1:2, :].to_broadcast((HD, SC)))
                nc.vector.tensor_mul(on_t[0:HD, :], ocA[0:HD, :], rb[0:HD, :])
                nc.vector.tensor_mul(on_t[HD:P, :], on_t[HD:P, :], rb[HD:P, :])

            def emit_kd(f, sp):
                ps = ps_op.tile([P, SC], F32, name="pskd", tag="op")
                for d in range(ND):
                    nc.tensor.matmul(
                        ps[:], wk[:, d * E + f * P:d * E + (f + 1) * P],
                        xsl(d, sp * SC, (sp + 1) * SC),
                        start=(d == 0), stop=(d == ND - 1))
                nc.vector.tensor_scalar_add(
                    kt[f][:, sp * SC:(sp + 1) * SC], ps[:], bkt[:, f:f + 1])

            def emit_q(sn, f):
                blk = (sn % 2) * QBLK
                ps = ps_op.tile([P, SC], F32, name="psqd", tag="op")
                for d in range(ND):
                    nc.tensor.matmul(
                        ps[:], wq[:, d * E + f * P:d * E + (f + 1) * P],
                        xsl(d, sn * SC, (sn + 1) * SC),
                        start=(d == 0), stop=(d == ND - 1))
                nc.vector.tensor_scalar_add(
                    qt2[f][0:HD, blk:blk + SC], ps[0:HD, :], bqt[0:HD, f:f + 1])
                nc.vector.tensor_scalar_add(
                    qt2[f][HD:P, blk + SC:blk + QBLK], ps[HD:P, :], bqt[HD:P, f:f + 1])

            def emit_outproj_dc(s, dc):
                on_tiles = on_s[s]
                op_ps = ps_op.tile([P, SC], F32, name="opps", tag="op")
                for e in range(NF):
                    nc.tensor.matmul(
                        op_ps[:], wo[:, e * D + dc * P:e * D + (dc + 1) * P],
                        on_tiles[e][:], start=(e == 0), stop=(e == NF - 1))
                ot = ot_pool.tile([P, SC], BF16, name="ottile", tag="ot")
                nc.vector.tensor_scalar_add(ot[:], op_ps[:], bot[:, dc:dc + 1])
                hc = SC // 2
                nc.sync.dma_start(
                    outT[dc * P:(dc + 1) * P, s * SC:s * SC + hc], ot[:, 0:hc])
                nc.sync.dma_start(
                    outT[dc * P:(dc + 1) * P, s * SC + hc:(s + 1) * SC], ot[:, hc:SC])

            cur_ps = [None]

            def emit_pv(s, hp, t, pt):
                if t == 0:
                    cur_ps[0] = (
                        ps_o.tile([P, SC], F32, name="opsA", tag="oA"),
                        ps_o.tile([P, SC], F32, name="opsB", tag="oB"))
                o_psA, o_psB = cur_ps[0]
                hA, hB = 2 * hp, 2 * hp + 1
                nc.tensor.matmul(
                    o_psA[0:HD1, :], vt[t][:, hA * HD1:(hA + 1) * HD1],
                    pt[:, 0:SC], start=(t == 0), stop=(t == NT - 1))
                nc.tensor.matmul(
                    o_psB[0:HD1, :], vt[t][:, hB * HD1:(hB + 1) * HD1],
                    pt[:, SC:2 * SC], start=(t == 0), stop=(t == NT - 1))
                if t == NT - 1:
                    emit_norm(s, hp, o_psA, o_psB)

            # deferred work: Q(s+1) mid-group; outproj(s) early in s+1
            due = {}

            def _idx(s, hp, t):
                return (s * NF + hp) * NT + t

            # K for f=1..3 streams just-in-time during the s0 groups f-1
            for f in range(1, NF):
                for sp in range(NS):
                    due.setdefault(_idx(0, f - 1, 2 + 3 * sp), []).append(
                        (emit_kd, (f, sp)))
            for s in range(NS - 1):
                for f in range(NF):
                    due.setdefault(_idx(s, f, 12), []).append(
                        (emit_q, (s + 1, f)))
                for dc in range(ND):
                    due.setdefault(_idx(s + 1, 0, 2) + 3 * dc, []).append(
                        (emit_outproj_dc, (s, dc)))

            pending = None
            for s in range(NS):
                blk = (s % 2) * QBLK
                for hp in range(NF):
                    for t in range(NT):
                        k = _idx(s, hp, t)
                        tsl = slice(t * P, (t + 1) * P)
                        sc_ps = ps_sc.tile([P, 2 * SC], F32, name="scps", tag="sc")
                        nc.tensor.matmul(
                            sc_ps[:, 0:SC], kt[hp][:, tsl],
                            qt2[hp][:, blk:blk + SC], start=True, stop=True)
                        nc.tensor.matmul(
                            sc_ps[:, SC:2 * SC], kt[hp][:, tsl],
                            qt2[hp][:, blk + SC:blk + QBLK], start=True, stop=True)
                        pt = pt_pool.tile([P, 2 * SC], BF16, name="ptile", tag="pt")
                        nc.scalar.activation(pt[:], sc_ps[:], EXP, scale=float(SCALE))
                        if pending is not None:
                            emit_pv(*pending)
                        pending = (s, hp, t, pt)
                        for fn, args in due.get(k, []):
                            fn(*args)
            emit_pv(*pending)
            for dc in range(ND):
                emit_outproj_dc(NS - 1, dc)

    nc.finalize()
    return nc


def _get_nc():
    if "nc" not in _NC_CACHE:
        _NC_CACHE["nc"] = _build_nc()
    return _NC_CACHE["nc"]


def _shard_inputs(x, w_qkv, b_qkv, w_out, b_out):
    """Build the 8 per-core input maps. Core i = (b = i//2, g = i%2)."""
    x = np.asarray(x, np.float32)
    w_qkv = np.asarray(w_qkv, np.float32)
    b_qkv = np.asarray(b_qkv, np.float32)
    w_out = np.asarray(w_out, np.float32)
    b_out = np.asarray(b_out, np.float32)

    BF = ml_dtypes.bfloat16
    in_maps = []
    for b in range(B):
        xT = np.ascontiguousarray(x[b].T.astype(BF))  # [D, S]
        for g in range(2):
            heads = range(g * HPG, (g + 1) * HPG)
            # w_qkv rows for head h: [192h, 192h+64) = Q, +64..128 = K, +128..192 = V
            q_rows = np.concatenate([np.arange(3 * HD * h, 3 * HD * h + HD) for h in heads])
            k_rows = q_rows + HD
            v_rows = q_rows + 2 * HD
            wqT = np.ascontiguousarray(w_qkv[q_rows].T.astype(BF))  # [D, E]
            wkT = np.ascontiguousarray(w_qkv[k_rows].T.astype(BF))
            wvT = np.ascontiguousarray(w_qkv[v_rows].T.astype(BF))
            ecols = np.arange(g * E, (g + 1) * E)
            woT = np.ascontiguousarray(w_out[:, ecols].T.astype(BF))  # [E, D]
            bo = b_out[:, None] if g == 0 else np.zeros((D, 1), np.float32)
            in_maps.append({
                "xT": xT,
                "wqT": wqT,
                "wkT": wkT,
                "wvT": wvT,
                "woT": woT,
                "bq": np.ascontiguousarray(b_qkv[q_rows][:, None]),
                "bk": np.ascontiguousarray(b_qkv[k_rows][:, None]),
                "bv": np.ascontiguousarray(b_qkv[v_rows][None, :]),
                "bo": np.ascontiguousarray(bo),
            })
    return in_maps


def run(inputs, trace=False):
    """Run the kernel; returns (full_output, exec_time_ns or None)."""
    nc = _get_nc()
    in_maps = _shard_inputs(**inputs)
    res = run_bass_kernel_spmd(nc, in_maps, core_ids=list(range(8)), trace=trace)
    out = np.empty((B, S, D), np.float32)
    for b in range(B):
        acc = (res.results[2 * b]["outT"].astype(np.float32)
               + res.results[2 * b + 1]["outT"].astype(np.float32))
        out[b] = acc.T
    return out, res.exec_time_ns


def kernel(x, w_qkv, b_qkv, w_out, b_out):
    out, _ = run(dict(x=x, w_qkv=w_qkv, b_qkv=b_qkv, w_out=w_out, b_out=b_out))
    return out


# revision 30
# speedup vs baseline: 1.1698x; 1.0122x over previous
"""Multi-head self-attention (B=4, S=2048, D=1024, H=16) on 8 TRN2 NeuronCores.

Sharding: core i = (batch b = i//2, head-group g = i%2). Each core computes,
for its batch and its 8 heads: QKV projection, attention, and a partial
output projection over its 512 attention features. Host sums the two
partials per batch (Megatron-style tensor parallel over heads x data
parallel over batch).

V5 notes (what matters on this hardware, measured):
  - ScalarE exp is the pacing engine: 256 x [128,1024] ACTIVATEs at
    ~1.11us + ~0.1us semaphore evaluation each = ~313us floor. Batching
    wider needs >8 PSUM banks. The attention loop is flattened over all
    (s, head-pair, t) steps with scores+exp issued one step ahead of the
    PV matmuls so the exp stream never waits on the in-order PE queue.
  - fp32 [128,128] stationary operands cannot double-buffer in the PE
    weight RAM, so their LDWEIGHTS serialize (~190ns each). Everything
    that loads stationary weights in the steady state (K^T tiles, V
    tiles, Wq, Wout) is bf16; P^T and On are bf16 so those matmuls are
    pure-bf16. PSUM accumulation stays fp32.
  - Every dma_start costs ~0.65us of serial sync-engine dispatch, so
    inputs are loaded with ONE dma_start per tensor (interleaved-AP
    scatter into a single wide SBUF tile), not per 128-row tile.
  - The softmax denominators bounce through DRAM to spread across
    partitions (engines cannot partition-broadcast); the chain is
    latency- not bandwidth-bound, so it is kept to 7 hops and head-B
    rows move to partitions 64-127 (unnormalized, bf16) in parallel.
  - fp8 was evaluated and rejected: attention output is a cancellation
    sum, so per-element fp8 error (~4%) survives as ~3-5e-2 output
    error (gate 2e-2). bf16 measures ~4-5e-3 end to end.

Per-core dataflow (transposed orientation so the softmax denominator
comes out of the PE array for free):
  V[t,e]   = x^T-stationary matmuls over Wv^T + ones column per head
  K^T[f,s] = Wk-stationary matmuls over x^T
  Q^T[f,s] = Wq-stationary, zero-padded per head half in a 2-deep ring
  S^T[t,s] = K^T-tile-stationary matmuls against qt2 halves
  P^T      = exp(S^T / 8) (ScalarE, PSUM->SBUF bf16; no max-sub needed)
  O^T_aug  = V_aug-stationary matmuls over P^T (M=65); row 64 = denom
  On = O^T * recip(denom); out^T = Wout^T-stationary over On (bf16
  partials, summed in fp32 on the host).
"""
import os
import sys
import types

import ml_dtypes
import numpy as np

# ---------------------------------------------------------------------------
# environment bootstrap (self-contained: no problem-dir imports)
# ---------------------------------------------------------------------------


def _install_ntff_hook():
    """run_bass_kernel_spmd(trace=True) under axon needs antenv.axon_hooks,
    which the agent image's antenv stub lacks. Recreate it."""
    if "antenv.axon_hooks" in sys.modules:
        return
    try:
        import antenv
        from trn_agent_boot.trn_boot import _ntff_profile_via_ctypes
    except Exception:
        return
    so_path = "/opt/axon/libaxon_pjrt.so"
    if not os.path.exists(so_path):
        return
    mod = types.ModuleType("antenv.axon_hooks")
    _hook = [_ntff_profile_via_ctypes(so_path)]
    mod.get_axon_ntff_profile_hook = lambda: _hook[0]

    def _set(h):
        _hook[0] = h

    mod.set_axon_ntff_profile_hook = _set
    sys.modules["antenv.axon_hooks"] = mod
    antenv.axon_hooks = mod


_install_ntff_hook()

import concourse.bacc as bacc
import concourse.tile as tile
from concourse import mybir
from concourse.bass_utils import run_bass_kernel_spmd
from contextlib import ExitStack

# ---------------------------------------------------------------------------
# problem constants (hardcoded per contract)
# ---------------------------------------------------------------------------
B, S, D = 4, 2048, 1024
H, HD = 16, 64
HPG = 8            # heads per core (group)
E = HPG * HD       # 512 attention features per core
P = 128
SC = 512           # s-chunk
NS = S // SC       # 4 s-chunks
NT = S // P        # 16 t-chunks
ND = D // P        # 8 d-chunks
NF = E // P        # 4 f-chunks per Q (or K) = head-pairs
HD1 = HD + 1       # V_aug columns per head (V + ones)
QBLK = 2 * SC      # one s-block in the qt2 ring
SCALE = 1.0 / np.sqrt(np.float32(HD))

F32 = mybir.dt.float32
F32R = mybir.dt.float32r
BF16 = mybir.dt.bfloat16
EXP = mybir.ActivationFunctionType.Exp

_NC_CACHE = {}


def _build_nc():
    nc = bacc.Bacc("TRN2", target_bir_lowering=False)

    xT = nc.dram_tensor("xT", [D, S], BF16, kind="ExternalInput")
    wqT = nc.dram_tensor("wqT", [D, E], BF16, kind="ExternalInput")
    wkT = nc.dram_tensor("wkT", [D, E], BF16, kind="ExternalInput")
    wvT = nc.dram_tensor("wvT", [D, E], BF16, kind="ExternalInput")
    woT = nc.dram_tensor("woT", [E, D], BF16, kind="ExternalInput")
    bq = nc.dram_tensor("bq", [E, 1], F32, kind="ExternalInput")
    bk = nc.dram_tensor("bk", [E, 1], F32, kind="ExternalInput")
    bv = nc.dram_tensor("bv", [1, E], F32, kind="ExternalInput")
    bo = nc.dram_tensor("bo", [D, 1], F32, kind="ExternalInput")
    outT = nc.dram_tensor("outT", [D, S], BF16, kind="ExternalOutput")

    with tile.TileContext(nc) as tc, ExitStack() as glob:
        const = glob.enter_context(tc.tile_pool(name="const", bufs=1))
        bv_bc = const.tile([P, E], F32, name="bv_bc")
        resid = glob.enter_context(tc.tile_pool(name="resid", bufs=1))
        # qt2[f]: 2-deep ring of s-blocks, each block [half, SC]: half 0
        # holds head-A rows 0-63 (64-127 zero), half 1 the opposite.
        qt2 = [resid.tile([P, 2 * QBLK], BF16, name=f"qt2_{f}") for f in range(NF)]
        kt = [resid.tile([P, S], BF16, name=f"kt{f}") for f in range(NF)]
        vt = [resid.tile([P, HPG * HD1], BF16, name=f"vt{t}") for t in range(NT)]
        xf = resid.tile([P, ND * S], BF16, name="xf")  # x^T, d-major blocks
        for f in range(NF):
            qv = qt2[f][:].rearrange("p (r h c) -> p r h c", h=2, c=SC)
            nc.vector.memset(qv[HD:P, :, 0, :], 0.0)
            nc.vector.memset(qv[0:HD, :, 1, :], 0.0)
        gw = glob.enter_context(tc.tile_pool(name="gw", bufs=1))
        wq = gw.tile([P, ND * E], BF16, name="wq")
        wk = gw.tile([P, ND * E], BF16, name="wk")
        bqt = gw.tile([P, NF], F32, name="bqt")
        bkt = gw.tile([P, NF], F32, name="bkt")

        def xsl(d, lo, hi):
            return xf[:, d * S + lo:d * S + hi]

        # ---------------- phase 1: V + K (+ Q for s0) --------------------
        with ExitStack() as c1:
            wpool = c1.enter_context(tc.tile_pool(name="w", bufs=1))
            wv = wpool.tile([P, ND * E], BF16, name="wv")
            # one dma_start per tensor (sync-engine dispatch is ~0.65us
            # each, serial); first-needed first.
            hd2 = ND // 2
            for h in range(2):
                dsl = slice(h * hd2, (h + 1) * hd2)
                rsl = slice(h * hd2 * P, (h + 1) * hd2 * P)
                nc.sync.dma_start(
                    wv[:].rearrange("p (d e) -> p d e", e=E)[:, dsl, :],
                    wvT[rsl, :].rearrange("(d p) e -> p d e", p=P))
                nc.sync.dma_start(
                    xf[:].rearrange("p (d c) -> p d c", c=S)[:, dsl, 0:SC],
                    xT[rsl, 0:SC].rearrange("(d p) c -> p d c", p=P))
            nc.sync.dma_start(
                wk[:].rearrange("p (d e) -> p d e", e=E),
                wkT[:].rearrange("(d p) e -> p d e", p=P))
            for s in range(1, NS):
                nc.sync.dma_start(
                    xf[:].rearrange("p (d c) -> p d c", c=S)[:, :, s * SC:(s + 1) * SC],
                    xT[:, s * SC:(s + 1) * SC].rearrange("(d p) c -> p d c", p=P))
            nc.sync.dma_start(
                wq[:].rearrange("p (d e) -> p d e", e=E),
                wqT[:].rearrange("(d p) e -> p d e", p=P))
            nc.sync.dma_start(bv_bc[:], bv[0:1, :].to_broadcast((P, E)))
            nc.sync.dma_start(bqt[:], bq[:].rearrange("(f p) a -> p (f a)", p=P))
            nc.sync.dma_start(bkt[:], bk[:].rearrange("(f p) a -> p (f a)", p=P))

            psv = c1.enter_context(tc.tile_pool(name="psv", bufs=2, space="PSUM"))
            psq = c1.enter_context(tc.tile_pool(name="psq", bufs=4, space="PSUM"))

            for s in range(NS):
                # V: x-stationary, stream Wv (out [t, 512 feats])
                for i in range(NS):
                    t = s * NS + i
                    ps = psv.tile([P, E], F32, name="psvt", tag="psv")
                    for d in range(ND):
                        nc.tensor.matmul(
                            ps[:], xsl(d, t * P, (t + 1) * P),
                            wv[:, d * E:(d + 1) * E],
                            start=(d == 0), stop=(d == ND - 1))
                    vdst = vt[t][:].rearrange("p (h c) -> p h c", c=HD1)
                    nc.vector.tensor_add(
                        vdst[:, :, 0:HD],
                        ps[:].rearrange("p (h c) -> p h c", c=HD),
                        bv_bc[:].rearrange("p (h c) -> p h c", c=HD))
                    nc.vector.memset(vdst[:, :, HD:HD1], 1.0)
                # K for f=0 only; f=1..3 stream inside the attention pipeline
                ps = psq.tile([P, SC], F32, name="pskt", tag="psq")
                for d in range(ND):
                    nc.tensor.matmul(
                        ps[:], wk[:, d * E:d * E + P],
                        xsl(d, s * SC, (s + 1) * SC),
                        start=(d == 0), stop=(d == ND - 1))
                nc.vector.tensor_scalar_add(
                    kt[0][:, s * SC:(s + 1) * SC], ps[:], bkt[:, 0:1])
            # Q for s0 (later s-chunks stream inside the attention pipeline)
            for f in range(NF):
                ps = psq.tile([P, SC], F32, name="psqt", tag="psq")
                for d in range(ND):
                    nc.tensor.matmul(
                        ps[:], wq[:, d * E + f * P:d * E + (f + 1) * P],
                        xsl(d, 0, SC), start=(d == 0), stop=(d == ND - 1))
                nc.vector.tensor_scalar_add(
                    qt2[f][0:HD, 0:SC], ps[0:HD, :], bqt[0:HD, f:f + 1])
                nc.vector.tensor_scalar_add(
                    qt2[f][HD:P, SC:QBLK], ps[HD:P, :], bqt[HD:P, f:f + 1])

        # ---------------- phase 2: flattened attention pipeline ----------
        with ExitStack() as c2:
            wo_pool = c2.enter_context(tc.tile_pool(name="wo", bufs=1))
            wo = wo_pool.tile([P, NF * D], BF16, name="wo")
            nc.sync.dma_start(
                wo[:].rearrange("p (e c) -> p e c", c=D),
                woT[:].rearrange("(e p) c -> p e c", p=P))
            bot = wo_pool.tile([P, ND], F32, name="bot")
            nc.sync.dma_start(bot[:], bo[:].rearrange("(i p) a -> p (i a)", p=P))

            dram_pool = c2.enter_context(tc.tile_pool(name="dramrs", bufs=3, space="DRAM"))
            pt_pool = c2.enter_context(tc.tile_pool(name="pt", bufs=8))
            on_pool = c2.enter_context(tc.tile_pool(name="on", bufs=2))
            rs_pool = c2.enter_context(tc.tile_pool(name="rs", bufs=3))
            rb_pool = c2.enter_context(tc.tile_pool(name="rb", bufs=3))
            ot_pool = c2.enter_context(tc.tile_pool(name="ot", bufs=4))
            ps_sc = c2.enter_context(tc.tile_pool(name="ps_sc", bufs=2, space="PSUM"))
            ps_o = c2.enter_context(tc.tile_pool(name="ps_o", bufs=1, space="PSUM"))
            ps_op = c2.enter_context(tc.tile_pool(name="ps_op", bufs=2, space="PSUM"))

            on_s = {}

            def alloc_on(s):
                if s not in on_s:
                    on_s[s] = [on_pool.tile([P, SC], BF16, name="on", tag=f"on{hp}")
                               for hp in range(NF)]
                return on_s[s]

            def emit_norm(s, hp, o_psA, o_psB):
                on_t = alloc_on(s)[hp]
                # tiny sums-row copies first so the reciprocal DMA chain
                # launches before the bulk evictions finish
                sA = rs_pool.tile([P, SC], F32, name="sA", tag="sA")
                sB = rs_pool.tile([P, SC], F32, name="sB", tag="sB")
                nc.vector.tensor_copy(sA[HD:HD1, :], o_psA[HD:HD1, :])
                nc.vector.tensor_copy(sB[HD:HD1, :], o_psB[HD:HD1, :])
                rd = dram_pool.tile([2, SC], F32, name="rdtile", tag="rd")
                nc.sync.dma_start(rd[0:1, :], sA[HD:HD1, :])
                nc.sync.dma_start(rd[1:2, :], sB[HD:HD1, :])
                ocA = rs_pool.tile([P, SC], F32, name="ocA", tag="ocA")
                ocB = rs_pool.tile([P, SC], BF16, name="ocB", tag="ocB")
                nc.vector.tensor_copy(ocA[0:HD, :], o_psA[0:HD, :])
                nc.vector.tensor_copy(ocB[0:HD, :], o_psB[0:HD, :])
                # head B rows move (unnormalized) to partitions 64-127,
                # overlapping the reciprocal chain; normalized in place.
                nc.sync.dma_start(on_t[HD:P, :], ocB[0:HD, :])
                # both sums rows spread over 128 partitions in one DMA
                rsp = rs_pool.tile([P, SC // HD], F32, name="rsp", tag="rsp")
                nc.sync.dma_start(
                    rsp[:], rd[0:2, :].rearrange("a (p c) -> (a p) c", c=SC // HD))
                rspb = rs_pool.tile([P, SC // HD], BF16, name="rspb", tag="rspb")
                with nc.allow_low_precision(reason="bf16 softmax recip broadcast"):
                    nc.vector.reciprocal(rspb[:], rsp[:])
                rdb = dram_pool.tile([2, SC], BF16, name="rdbtile", tag="rdb")
                nc.sync.dma_start(
                    rdb[0:2, :].rearrange("a (p c) -> (a p) c", c=SC // HD),
                    rspb[:])
                rb = rb_pool.tile([P, SC], BF16, name="rbtile", tag="rb")
                nc.sync.dma_start(rb[0:HD, :], rdb[0:1, :].to_broadcast((HD, SC)))
                nc.sync.dma_start(rb[HD:P, :], rdb[1:2, :].to_broadcast((HD, SC)))
                nc.vector.tensor_mul(on_t[0:HD, :], ocA[0:HD, :], rb[0:HD, :])
                nc.vector.tensor_mul(on_t[HD:P, :], on_t[HD:P, :], rb[HD:P, :])

            def emit_kd(f, sp):
                ps = ps_op.tile([P, SC], F32, name="pskd", tag="op")
                for d in range(ND):
                    nc.tensor.matmul(
                        ps[:], wk[:, d * E + f * P:d * E + (f + 1) * P],
                        xsl(d, sp * SC, (sp + 1) * SC),
                        start=(d == 0), stop=(d == ND - 1))
                nc.vector.tensor_scalar_add(
                    kt[f][:, sp * SC:(sp + 1) * SC], ps[:], bkt[:, f:f + 1])

            def emit_q(sn, f):
                blk = (sn % 2) * QBLK
                ps = ps_op.tile([P, SC], F32, name="psqd", tag="op")
                for d in range(ND):
                    nc.tensor.matmul(
                        ps[:], wq[:, d * E + f * P:d * E + (f + 1) * P],
                        xsl(d, sn * SC, (sn + 1) * SC),
                        start=(d == 0), stop=(d == ND - 1))
                nc.vector.tensor_scalar_add(
                    qt2[f][0:HD, blk:blk + SC], ps[0:HD, :], bqt[0:HD, f:f + 1])
                nc.vector.tensor_scalar_add(
                    qt2[f][HD:P, blk + SC:blk + QBLK], ps[HD:P, :], bqt[HD:P, f:f + 1])

            def emit_outproj_dc(s, dc):
                on_tiles = on_s[s]
                op_ps = ps_op.tile([P, SC], F32, name="opps", tag="op")
                for e in range(NF):
                    nc.tensor.matmul(
                        op_ps[:], wo[:, e * D + dc * P:e * D + (dc + 1) * P],
                        on_tiles[e][:], start=(e == 0), stop=(e == NF - 1))
                ot = ot_pool.tile([P, SC], BF16, name="ottile", tag="ot")
                nc.vector.tensor_scalar_add(ot[:], op_ps[:], bot[:, dc:dc + 1])
                nc.sync.dma_start(
                    outT[dc * P:(dc + 1) * P, s * SC:(s + 1) * SC], ot[:])

            cur_ps = [None]

            def emit_pv(s, hp, t, pt):
                if t == 0:
                    cur_ps[0] = (
                        ps_o.tile([P, SC], F32, name="opsA", tag="oA"),
                        ps_o.tile([P, SC], F32, name="opsB", tag="oB"))
                o_psA, o_psB = cur_ps[0]
                hA, hB = 2 * hp, 2 * hp + 1
                nc.tensor.matmul(
                    o_psA[0:HD1, :], vt[t][:, hA * HD1:(hA + 1) * HD1],
                    pt[:, 0:SC], start=(t == 0), stop=(t == NT - 1))
                nc.tensor.matmul(
                    o_psB[0:HD1, :], vt[t][:, hB * HD1:(hB + 1) * HD1],
                    pt[:, SC:2 * SC], start=(t == 0), stop=(t == NT - 1))
                if t == NT - 1:
                    emit_norm(s, hp, o_psA, o_psB)

            # deferred work: Q(s+1) mid-group; outproj(s) early in s+1
            due = {}

            def _idx(s, hp, t):
                return (s * NF + hp) * NT + t

            # K for f=1..3 streams just-in-time during the s0 groups f-1
            for f in range(1, NF):
                for sp in range(NS):
                    due.setdefault(_idx(0, f - 1, 2 + 3 * sp), []).append(
                        (emit_kd, (f, sp)))
            # Q(s1) in the (s0, hp3) group (the only s0 group without K work)
            for f in range(NF):
                due.setdefault(_idx(0, 3, 2 + 3 * f), []).append(
                    (emit_q, (1, f)))
            for s in range(1, NS - 1):
                for f in range(NF):
                    due.setdefault(_idx(s, f, 13), []).append(
                        (emit_q, (s + 1, f)))
            # outproj(s): two dc chunks per group of s+1
            for s in range(NS - 1):
                for dc in range(ND):
                    due.setdefault(_idx(s + 1, dc // 2, 2 + 6 * (dc % 2)), []).append(
                        (emit_outproj_dc, (s, dc)))

            pending = None
            for s in range(NS):
                blk = (s % 2) * QBLK
                for hp in range(NF):
                    for t in range(NT):
                        k = _idx(s, hp, t)
                        tsl = slice(t * P, (t + 1) * P)
                        sc_ps = ps_sc.tile([P, 2 * SC], F32, name="scps", tag="sc")
                        nc.tensor.matmul(
                            sc_ps[:, 0:SC], kt[hp][:, tsl],
                            qt2[hp][:, blk:blk + SC], start=True, stop=True)
                        nc.tensor.matmul(
                            sc_ps[:, SC:2 * SC], kt[hp][:, tsl],
                            qt2[hp][:, blk + SC:blk + QBLK], start=True, stop=True)
                        pt = pt_pool.tile([P, 2 * SC], BF16, name="ptile", tag="pt")
                        nc.scalar.activation(pt[:], sc_ps[:], EXP, scale=float(SCALE))
                        if pending is not None:
                            emit_pv(*pending)
                        pending = (s, hp, t, pt)
                        for fn, args in due.get(k, []):
                            fn(*args)
            emit_pv(*pending)
            for dc in range(ND):
                emit_outproj_dc(NS - 1, dc)

    nc.finalize()
    return nc


def _get_nc():
    if "nc" not in _NC_CACHE:
        _NC_CACHE["nc"] = _build_nc()
    return _NC_CACHE["nc"]


def _shard_inputs(x, w_qkv, b_qkv, w_out, b_out):
    """Build the 8 per-core input maps. Core i = (b = i//2, g = i%2)."""
    x = np.asarray(x, np.float32)
    w_qkv = np.asarray(w_qkv, np.float32)
    b_qkv = np.asarray(b_qkv, np.float32)
    w_out = np.asarray(w_out, np.float32)
    b_out = np.asarray(b_out, np.float32)

    BF = ml_dtypes.bfloat16
    in_maps = []
    for b in range(B):
        xT = np.ascontiguousarray(x[b].T.astype(BF))  # [D, S]
        for g in range(2):
            heads = range(g * HPG, (g + 1) * HPG)
            # w_qkv rows for head h: [192h, 192h+64) = Q, +64..128 = K, +128..192 = V
            q_rows = np.concatenate([np.arange(3 * HD * h, 3 * HD * h + HD) for h in heads])
            k_rows = q_rows + HD
            v_rows = q_rows + 2 * HD
            wqT = np.ascontiguousarray(w_qkv[q_rows].T.astype(BF))  # [D, E]
            wkT = np.ascontiguousarray(w_qkv[k_rows].T.astype(BF))
            wvT = np.ascontiguousarray(w_qkv[v_rows].T.astype(BF))
            ecols = np.arange(g * E, (g + 1) * E)
            woT = np.ascontiguousarray(w_out[:, ecols].T.astype(BF))  # [E, D]
            bo = b_out[:, None] if g == 0 else np.zeros((D, 1), np.float32)
            in_maps.append({
                "xT": xT,
                "wqT": wqT,
                "wkT": wkT,
                "wvT": wvT,
                "woT": woT,
                "bq": np.ascontiguousarray(b_qkv[q_rows][:, None]),
                "bk": np.ascontiguousarray(b_qkv[k_rows][:, None]),
                "bv": np.ascontiguousarray(b_qkv[v_rows][None, :]),
                "bo": np.ascontiguousarray(bo),
            })
    return in_maps


def run(inputs, trace=False):
    """Run the kernel; returns (full_output, exec_time_ns or None)."""
    nc = _get_nc()
    in_maps = _shard_inputs(**inputs)
    res = run_bass_kernel_spmd(nc, in_maps, core_ids=list(range(8)), trace=trace)
    out = np.empty((B, S, D), np.float32)
    for b in range(B):
        acc = (res.results[2 * b]["outT"].astype(np.float32)
               + res.results[2 * b + 1]["outT"].astype(np.float32))
        out[b] = acc.T
    return out, res.exec_time_ns


def kernel(x, w_qkv, b_qkv, w_out, b_out):
    out, _ = run(dict(x=x, w_qkv=w_qkv, b_qkv=b_qkv, w_out=w_out, b_out=b_out))
    return out
